# revision 73
# baseline (speedup 1.0000x reference)
"""DGM kNN kernel for Trainium2 (Bass/Tile), SPMD over 8 NeuronCores.

Problem: 3-layer MLP on x[4,4096,64], pairwise sq-distances per batch,
top-k=8 smallest per row (scaled by tau), outputs (edges, logprobs).

Sharding: core c handles batch c//2, query rows (c%2)*2048 ..+2048.
Each core computes the full-batch MLP (keys) + its query half, the
[2048, 4096] scaled-distance block, and native max8/max_index top-8.
"""

import numpy as np

import concourse.bass as bass
import concourse.mybir as mybir
import concourse.tile as tile
from concourse.bass_utils import run_bass_kernel_spmd

F32 = mybir.dt.float32
F32R = mybir.dt.float32r
U32 = mybir.dt.uint32
AF = mybir.ActivationFunctionType

B, N, D, K = 4, 4096, 64, 8
NCORES = 8
NQ = N // 2          # query rows per core
P = 128
NT = N // P          # 32 x-tiles (keys)
NTQ = NQ // P        # 16 x-tiles (queries) == q tiles
MM_F = 512           # matmul moving free dim (one PSUM bank of fp32)

_prog_cache = {}


def _build_program():
    nc = bass.Bass("TRN2")
    # reserved before TileContext so the numbers never collide with tile sems;
    # used by _legalize_matmul_waits as no-op update targets (one per engine
    # so the race detector sees a single updater per sem)
    nc._legalize_dummy_sems = {
        eng: nc.alloc_semaphore(name=f"legalize_dummy_{eng.name}")
        for eng in (
            mybir.EngineType.PE,
            mybir.EngineType.Activation,
            mybir.EngineType.DVE,
            mybir.EngineType.Pool,
            mybir.EngineType.SP,
        )
    }

    # keys arrive pre-transposed AND rolled per core so that this core's
    # query block is always key columns [0:NQ] (host un-rolls indices).
    # x and W come pre-split into f32r hi/lo pairs (exact decomposition),
    # stacked along the contract dim: [hi(64); lo(64)].
    xhl_d = nc.dram_tensor("xhl", [2 * D, N], F32R, kind="ExternalInput")
    # per layer: diag stack [Whi; Wlo] and cross stack [Wlo; Whi]
    wd_d = [nc.dram_tensor(f"wd{i}", [2 * D, D], F32R, kind="ExternalInput")
            for i in range(3)]
    wc_d = [nc.dram_tensor(f"wc{i}", [2 * D, D], F32R, kind="ExternalInput")
            for i in range(3)]
    # bias columns: b1, b2, b3, -2*b3
    bias_d = nc.dram_tensor("bias", [D, 4], F32, kind="ExternalInput")
    # -tau broadcast per partition
    negtau_d = nc.dram_tensor("negtau", [P, 1], F32, kind="ExternalInput")

    val_out = nc.dram_tensor("val_out", [NQ, K], F32, kind="ExternalOutput")
    idx_out = nc.dram_tensor("idx_out", [NQ, K], U32, kind="ExternalOutput")

    with tile.TileContext(nc) as tc:
        with (
            tc.tile_pool(name="consts", bufs=1) as consts,
            tc.tile_pool(name="mlp", bufs=1) as mlp_pool,
            tc.tile_pool(name="scaled", bufs=3) as sc_pool,
            tc.tile_pool(name="outs", bufs=1) as out_pool,
        ):
            wd, wc = [], []
            for i in range(3):
                w1 = consts.tile([2 * D, D], F32R, tag=f"wd{i}")
                nc.sync.dma_start(out=w1[:, :], in_=wd_d[i][:, :])
                wd.append(w1)
                w2 = consts.tile([2 * D, D], F32R, tag=f"wc{i}")
                nc.sync.dma_start(out=w2[:, :], in_=wc_d[i][:, :])
                wc.append(w2)
            bias_sb = consts.tile([D, 4], F32)
            nc.sync.dma_start(out=bias_sb[:, :], in_=bias_d[:, :])
            negtau = consts.tile([P, 1], F32)
            nc.sync.dma_start(out=negtau[:, :], in_=negtau_d[:, :])

            ones64 = consts.tile([D, 1], F32)
            nc.vector.memset(ones64[:, :], 1.0)
            ones64_2 = consts.tile([D, 2], F32)
            nc.vector.memset(ones64_2[:, :], 1.0)
            ones64r = consts.tile([D, 2], F32R)
            nc.vector.tensor_copy(ones64r[:, :], ones64_2[:, :])

            # ---- x arrives pre-transposed/rolled/hi-lo-stacked: [128, n] ----
            xhl = mlp_pool.tile([2 * D, N], F32R, tag="hl_a")
            nc.sync.dma_start(out=xhl[:, :], in_=xhl_d[:, :])

            # ---- 3-layer MLP via f32r hi/lo sweeps (keys only) ----
            # Intermediate hi/lo stay on lanes 0..63 -- both Whi and Wlo
            # exist at both lane ranges via wd/wc slices, so no partition
            # moves are ever needed.
            rhs65 = mlp_pool.tile([D + 1, N], F32, tag="rhs65")
            hmid = mlp_pool.tile([D, N], F32, tag="hmid")
            lo64 = mlp_pool.tile([D, N], F32R, tag="lo64")
            hl_b = mlp_pool.tile([2 * D, N], F32R, tag="hl_b")

            with tc.tile_pool(name="ps_mlp", bufs=1, space="PSUM") as ps_mlp_pool:
                cur = xhl
                for layer in range(3):
                    last = layer == 2
                    pss = []
                    for stat in (wd[layer], wc[layer]):
                        for ci, c in enumerate(range(0, N, MM_F)):
                            if stat is wd[layer]:
                                ps = ps_mlp_pool.tile([D, MM_F], F32, tag=f"m{ci}")
                                pss.append(ps)
                            nc.tensor.matmul(
                                pss[ci][:, :],
                                lhsT=stat[:, :],
                                rhs=cur[:, c:c + MM_F],
                                start=stat is wd[layer],
                                stop=stat is wc[layer],
                            )
                    dst = rhs65 if last else hmid
                    nxt_hl = hl_b if layer == 0 else xhl
                    # chunk-wise drain + hi/lo re-split so the next layer's
                    # matmuls start while later chunks are still splitting
                    for ci, c in enumerate(range(0, N, MM_F)):
                        sl = slice(c, c + MM_F)
                        nc.scalar.activation(
                            out=dst[0:D, sl],
                            in_=pss[ci][:, :],
                            func=AF.Identity if last else AF.Relu,
                            bias=bias_sb[:, (2 if last else layer):
                                         (3 if last else layer + 1)],
                            scale=1.0,
                        )
                        if not last:
                            nc.scalar.activation(
                                out=nxt_hl[0:D, sl], in_=hmid[:, sl], func=AF.Copy
                            )
                            nc.vector.tensor_sub(
                                lo64[:, sl], hmid[:, sl], nxt_hl[0:D, sl]
                            )
                            nc.sync.dma_start(
                                out=nxt_hl[D:2 * D, sl], in_=lo64[:, sl]
                            )
                    if not last:
                        cur = nxt_hl

            # ---- squared norms (keys; queries are a prefix slice) ----
            # sq = ones @ (htsq_hi + htsq_lo): exact f32r split of h*h, both
            # sweeps share the ones stationary (no weight-reload penalty)
            htsq = mlp_pool.tile([D, N], F32, tag="hmid")     # reuse
            nc.vector.tensor_mul(htsq[:, :], rhs65[0:D, :], rhs65[0:D, :])
            htsq_hi = mlp_pool.tile([D, N], F32R, tag="hl_b")  # reuse
            nc.scalar.activation(out=htsq_hi[:, :], in_=htsq[:, :], func=AF.Copy)
            htsq_lo = mlp_pool.tile([D, N], F32R, tag="lo64")  # reuse
            nc.vector.tensor_sub(htsq_lo[:, :], htsq[:, :], htsq_hi[:, :])

            sqrow0 = mlp_pool.tile([1, N], F32, tag="hmid")  # htsq fully read
            with tc.tile_pool(name="ps_sq", bufs=1, space="PSUM") as ps_sq_pool:
                pss = []
                for sweep, src in ((0, htsq_hi), (1, htsq_lo)):
                    for ci, c in enumerate(range(0, N, MM_F)):
                        if sweep == 0:
                            ps = ps_sq_pool.tile([2, MM_F], F32, tag=f"sq{ci}")
                            pss.append(ps)
                        nc.tensor.matmul(
                            pss[ci][0:2, :],
                            lhsT=ones64r[:, :],
                            rhs=src[:, c:c + MM_F],
                            start=sweep == 0, stop=sweep == 1,
                        )
                for ci, c in enumerate(range(0, N, MM_F)):
                    nc.scalar.activation(
                        out=sqrow0[0:1, c:c + MM_F],
                        in_=pss[ci][0:1, :],
                        func=AF.Copy,
                    )

            # ---- f32r hi/lo split of h for the fast distance matmuls ----
            # f32r keeps 11 mantissa bits; products of f32r inputs are exact
            # and accumulate in fp32 PSUM. Dropping the ~4e-6 lo@lo term lets
            # sq ride in a 66-contract second matmul (8 matmuls/half, not 12):
            #   MM1: [-2hi_q; -2lo_q] x [hi_k; hi_k]      (hihi + lohi)
            #   MM2: [-2hi_q; 1; 1]   x [lo_k; sqhi; sqlo] (hilo + sq_j)
            rhs_c = mlp_pool.tile([P, N], F32R, tag="rhs_b")    # [hi_k; hi_k]
            nc.vector.tensor_copy(rhs_c[0:D, :], rhs65[0:D, :])
            nc.sync.dma_start(out=rhs_c[D:P, :], in_=rhs_c[0:D, :])
            rhs_d = mlp_pool.tile([D + 2, N], F32R, tag="sq2")
            nc.vector.tensor_sub(rhs_d[0:D, :], rhs65[0:D, :], rhs_c[0:D, :])

            # sq_hi/sq_lo on lane 0, then DMA to rhs_d rows 64/65
            sqhi0 = mlp_pool.tile([1, N], F32R, tag="hl_b")  # htsq_hi done
            nc.scalar.activation(out=sqhi0[0:1, :], in_=sqrow0[0:1, :], func=AF.Copy)
            sqlo = mlp_pool.tile([1, N], F32R, tag="lo64")
            nc.vector.tensor_sub(sqlo[0:1, :], sqrow0[0:1, :], sqhi0[0:1, :])
            nc.sync.dma_start(out=rhs_d[D:D + 1, :], in_=sqhi0[0:1, :])
            nc.sync.dma_start(out=rhs_d[D + 1:D + 2, :], in_=sqlo[0:1, :])

            # query-side stationary tiles (queries = key cols [0:NQ]):
            # lhs_1 rows [-2hi_q; -2lo_q], lhs_3 rows [-2hi_q; 1; 1]
            lhs_1 = mlp_pool.tile([P, NQ], F32R, tag="lhs_1")
            lhs_3 = mlp_pool.tile([D + 2, NQ], F32R, tag="lhs_2")
            nc.scalar.activation(
                out=lhs_1[0:D, :], in_=rhs_c[0:D, 0:NQ],
                func=AF.Identity, bias=0.0, scale=-2.0,
            )
            nc.scalar.activation(
                out=lhs_3[0:D, :], in_=rhs_c[0:D, 0:NQ],
                func=AF.Identity, bias=0.0, scale=-2.0,
            )
            # -2*lo_q scratch on its own slot so the lhs_1 DMA does not
            # serialize behind unrelated readers of shared slots
            neg2lo = mlp_pool.tile([D, NQ], F32R, tag="neg2lo")
            nc.scalar.activation(
                out=neg2lo[:, :], in_=rhs_d[0:D, 0:NQ],
                func=AF.Identity, bias=0.0, scale=-2.0,
            )
            nc.sync.dma_start(out=lhs_1[D:P, :], in_=neg2lo[:, :])
            # ones rows 64..65 of lhs_3 via fp32-bitcast scratch in dead xhl
            nc.vector.memset(xhl[D:D + 2, 0:NQ].bitcast(F32), 1.0)
            nc.vector.tensor_copy(
                lhs_3[D:D + 2, :], xhl[D:D + 2, 0:NQ].bitcast(F32)
            )

            # per-query bias: -tau * sq_q where sq_q = sq_keys[0:NQ].
            # SBUF->SBUF partition scatter can't be balanced in one DMA, so
            # bounce through a small DRAM scratch tensor.
            sqq_dram = nc.dram_tensor("sqq_scratch", [NQ], F32, kind="Internal")
            nc.sync.dma_start(out=sqq_dram[:], in_=sqrow0[0:1, 0:NQ])
            sqt = consts.tile([P, NTQ], F32)
            nc.sync.dma_start(
                out=sqt[:, :],
                in_=sqq_dram[:].rearrange("(t p) -> p t", p=P),
            )
            bias_all = consts.tile([P, NTQ], F32)
            nc.vector.tensor_scalar_mul(bias_all[:, :], sqt[:, :], negtau[:, 0:1])

            # ---- distance blocks + top-8 ----
            vals_sb = out_pool.tile([P, NTQ * K], F32)
            idx_sb = out_pool.tile([P, NTQ * K], U32)

            with tc.tile_pool(name="ps_dist", bufs=2, space="PSUM") as ps_dist_pool:
                for t in range(NTQ):
                    sc = sc_pool.tile([P, N], F32, tag="sc")
                    for h in range(2):
                        ph = ps_dist_pool.tile([P, N // 2], F32, tag="dist")
                        # sweep each stationary over all 4 chunks before
                        # switching: weight reloads cost ~450ns, sweeps ~250ns
                        for phase, (lhsT, rhs) in enumerate((
                            (lhs_1[:, t * P:(t + 1) * P], rhs_c),
                            (lhs_3[:, t * P:(t + 1) * P], rhs_d),
                        )):
                            for i in range(4):
                                off = h * (N // 2) + i * MM_F
                                nc.tensor.matmul(
                                    ph[:, i * MM_F:(i + 1) * MM_F],
                                    lhsT=lhsT,
                                    rhs=rhs[:, off:off + MM_F],
                                    start=phase == 0, stop=phase == 1,
                                )
                        # scaled = -tau * (sq_j - 2G) - tau*sq_q
                        nc.scalar.activation(
                            out=sc[:, h * (N // 2):(h + 1) * (N // 2)],
                            in_=ph[:, :],
                            func=AF.Identity,
                            bias=bias_all[:, t:t + 1],
                            scale=negtau[:, 0:1],
                        )
                    nc.vector.max(
                        out=vals_sb[:, t * K:(t + 1) * K], in_=sc[:, :]
                    )
                    nc.vector.max_index(
                        out=idx_sb[:, t * K:(t + 1) * K],
                        in_max=vals_sb[:, t * K:(t + 1) * K],
                        in_values=sc[:, :],
                    )

            nc.sync.dma_start(
                out=val_out[:, :].rearrange("(t p) k -> p t k", p=P),
                in_=vals_sb[:, :].rearrange("p (t k) -> p t k", k=K),
            )
            nc.sync.dma_start(
                out=idx_out[:, :].rearrange("(t p) k -> p t k", p=P),
                in_=idx_sb[:, :].rearrange("p (t k) -> p t k", k=K),
            )

    return nc


_NO_HOIST = {"InstEventSemaphore"}


def _legalize_matmul_waits(nc, max_waits=1):
    """Walrus codegen rejects engine instructions with >1 embedded sync wait
    ("Too many sync wait commands"). Hoist excess waits onto standalone
    InstEventSemaphore instructions on the same engine just before the
    instruction (engines execute their stream in order, so semantics are
    preserved)."""
    n_fix = 0
    if not hasattr(nc, "_legalize_dummy_sems"):
        # pick sem numbers that no instruction in the program references
        used = set()
        for f in nc.m.functions:
            for blk in f.blocks:
                for inst in blk.instructions:
                    si = inst.sync_info
                    if si is not None:
                        used.update(w.id for w in si.on_wait)
                        used.update(u.id for u in si.on_update)
        engs = (
            mybir.EngineType.PE,
            mybir.EngineType.Activation,
            mybir.EngineType.DVE,
            mybir.EngineType.Pool,
            mybir.EngineType.SP,
        )
        free = [n for n in range(190, 100, -1) if n not in used]
        nc._legalize_dummy_sems = {
            eng: type("S", (), {"num": free[i], "name": f"lg_dummy_{eng.name}"})()
            for i, eng in enumerate(engs)
        }
    for f in nc.m.functions:
        for blk in f.blocks:
            out = []
            for inst in blk.instructions:
                si = inst.sync_info
                if (
                    type(inst).__name__ not in _NO_HOIST
                    and si is not None
                    and len(si.on_wait) > max_waits
                ):
                    waits = list(si.on_wait)
                    keep, hoist = waits[-max_waits:], waits[:-max_waits]
                    dummy = nc._legalize_dummy_sems[inst.engine]
                    for j, w in enumerate(hoist):
                        upd = mybir.SyncUpdate(
                            sync_type="semaphore",
                            id=dummy.num,
                            ant_name=dummy.name,
                            update_mode="sem-inc",
                            update_value=1,
                        )
                        ev = mybir.InstEventSemaphore(
                            name=f"EVW-{inst.name}-{j}",
                            engine=inst.engine,
                            ins=[],
                            outs=[],
                            sync_info=mybir.SyncInfo(on_wait=[w], on_update=[upd]),
                        )
                        out.append(ev)
                    inst.sync_info = mybir.SyncInfo(
                        on_wait=keep, on_update=list(si.on_update)
                    )
                    n_fix += 1
                out.append(inst)
            blk.instructions[:] = out
    return n_fix


def get_program():
    if "nc" not in _prog_cache:
        nc = _build_program()
        _legalize_matmul_waits(nc)
        _prog_cache["nc"] = nc
    return _prog_cache["nc"]


def make_in_maps(x, W1, b1, W2, b2, W3, b3, temperature):
    x = np.ascontiguousarray(np.asarray(x, dtype=np.float32))
    tau = np.exp(np.clip(np.float32(temperature), np.float32(-5.0),
                         np.float32(5.0)), dtype=np.float32)
    negtau = np.full((P, 1), -tau, dtype=np.float32)
    bias = np.stack(
        [np.asarray(b1, np.float32), np.asarray(b2, np.float32),
         np.asarray(b3, np.float32),
         np.float32(-2.0) * np.asarray(b3, np.float32)],
        axis=1,
    )
    bias = np.ascontiguousarray(bias)

    def trunc11(a):
        """Exact f32r split: hi keeps 11 explicit mantissa bits."""
        b = a.astype(np.float32).view(np.uint32)
        return (b & np.uint32(0xFFFFF000)).view(np.float32)

    def hl_stack(a, rev=False):
        hi = trunc11(a)
        lo = (a - hi).astype(np.float32)
        pair = (lo, hi) if rev else (hi, lo)
        return np.ascontiguousarray(np.concatenate(pair, axis=0))

    wts = {}
    for i, W in enumerate((W1, W2, W3)):
        wt = np.asarray(W, np.float32).T
        wts[f"wd{i}"] = hl_stack(wt)
        wts[f"wc{i}"] = hl_stack(wt, rev=True)

    in_maps = []
    xt_all = [np.ascontiguousarray(x[b_i].T) for b_i in range(B)]
    for c in range(NCORES):
        b_i, half = c // 2, c % 2
        xbt = xt_all[b_i]
        if half:
            # roll keys so this core's queries are key columns [0:NQ];
            # local key j holds global key (j + NQ) % N
            xbt = np.ascontiguousarray(np.roll(xbt, -NQ, axis=1))
        in_maps.append({
            "xhl": hl_stack(xbt),
            "bias": bias,
            "negtau": negtau,
            **wts,
        })
    return in_maps


def assemble_outputs(results):
    """results: list of 8 dicts with val_out [NQ,K] f32, idx_out [NQ,K] u32."""
    edges = np.empty((B, N * K, 2), dtype=np.int32)
    logprobs = np.empty((B, N, K), dtype=np.float32)
    rows = np.repeat(np.arange(N, dtype=np.int32), K)
    for b_i in range(B):
        idx = np.concatenate(
            [results[2 * b_i]["idx_out"].astype(np.int32),
             # second half-core saw keys rolled by NQ: un-roll indices
             (results[2 * b_i + 1]["idx_out"].astype(np.int32) + NQ) % N],
            axis=0,
        )
        vals = np.concatenate(
            [results[2 * b_i]["val_out"], results[2 * b_i + 1]["val_out"]], axis=0
        )
        edges[b_i, :, 0] = idx.reshape(-1)
        edges[b_i, :, 1] = rows
        logprobs[b_i] = vals
    return edges, logprobs


def run(inputs, trace=False):
    """Full pipeline; returns ((edges, logprobs), BassKernelResults)."""
    k = int(np.asarray(inputs["k"]))
    assert k == K, f"kernel hardcodes k=8, got {k}"
    nc = get_program()
    in_maps = make_in_maps(
        inputs["x"], inputs["W1"], inputs["b1"], inputs["W2"], inputs["b2"],
        inputs["W3"], inputs["b3"], inputs["temperature"],
    )
    br = run_bass_kernel_spmd(nc, in_maps, list(range(NCORES)), trace=trace)
    return assemble_outputs(br.results), br


def kernel(**inputs):
    (edges, logprobs), _ = run(inputs, trace=False)
    return edges, logprobs


# revision 74
# speedup vs baseline: 1.0061x; 1.0061x over previous
"""DGM kNN kernel for Trainium2 (Bass/Tile), SPMD over 8 NeuronCores.

Problem: 3-layer MLP on x[4,4096,64], pairwise sq-distances per batch,
top-k=8 smallest per row (scaled by tau), outputs (edges, logprobs).

Sharding: core c handles batch c//2, query rows (c%2)*2048 ..+2048.
Each core computes the full-batch MLP (keys) + its query half, the
[2048, 4096] scaled-distance block, and native max8/max_index top-8.
"""

import numpy as np

import concourse.bass as bass
import concourse.mybir as mybir
import concourse.tile as tile
from concourse.bass_utils import run_bass_kernel_spmd

F32 = mybir.dt.float32
F32R = mybir.dt.float32r
U32 = mybir.dt.uint32
AF = mybir.ActivationFunctionType

B, N, D, K = 4, 4096, 64, 8
NCORES = 8
NQ = N // 2          # query rows per core
P = 128
NT = N // P          # 32 x-tiles (keys)
NTQ = NQ // P        # 16 x-tiles (queries) == q tiles
MM_F = 512           # matmul moving free dim (one PSUM bank of fp32)

_prog_cache = {}


def _build_program():
    nc = bass.Bass("TRN2")
    # reserved before TileContext so the numbers never collide with tile sems;
    # used by _legalize_matmul_waits as no-op update targets (one per engine
    # so the race detector sees a single updater per sem)
    nc._legalize_dummy_sems = {
        eng: nc.alloc_semaphore(name=f"legalize_dummy_{eng.name}")
        for eng in (
            mybir.EngineType.PE,
            mybir.EngineType.Activation,
            mybir.EngineType.DVE,
            mybir.EngineType.Pool,
            mybir.EngineType.SP,
        )
    }

    # keys arrive pre-transposed AND rolled per core so that this core's
    # query block is always key columns [0:NQ] (host un-rolls indices).
    # x and W come pre-split into f32r hi/lo pairs (exact decomposition),
    # stacked along the contract dim: [hi(64); lo(64)].
    xhl_d = nc.dram_tensor("xhl", [2 * D, N], F32R, kind="ExternalInput")
    # per layer: diag stack [Whi; Wlo] and cross stack [Wlo; Whi]
    wd_d = [nc.dram_tensor(f"wd{i}", [2 * D, D], F32R, kind="ExternalInput")
            for i in range(3)]
    wc_d = [nc.dram_tensor(f"wc{i}", [2 * D, D], F32R, kind="ExternalInput")
            for i in range(3)]
    # bias columns: b1, b2, b3, -2*b3
    bias_d = nc.dram_tensor("bias", [D, 4], F32, kind="ExternalInput")
    # -tau broadcast per partition
    negtau_d = nc.dram_tensor("negtau", [P, 1], F32, kind="ExternalInput")

    val_out = nc.dram_tensor("val_out", [NQ, K], F32, kind="ExternalOutput")
    idx_out = nc.dram_tensor("idx_out", [NQ, K], U32, kind="ExternalOutput")

    with tile.TileContext(nc) as tc:
        with (
            tc.tile_pool(name="consts", bufs=1) as consts,
            tc.tile_pool(name="mlp", bufs=1) as mlp_pool,
            tc.tile_pool(name="scaled", bufs=3) as sc_pool,
            tc.tile_pool(name="outs", bufs=1) as out_pool,
        ):
            wd, wc = [], []
            for i in range(3):
                w1 = consts.tile([2 * D, D], F32R, tag=f"wd{i}")
                nc.sync.dma_start(out=w1[:, :], in_=wd_d[i][:, :])
                wd.append(w1)
                w2 = consts.tile([2 * D, D], F32R, tag=f"wc{i}")
                nc.sync.dma_start(out=w2[:, :], in_=wc_d[i][:, :])
                wc.append(w2)
            bias_sb = consts.tile([D, 4], F32)
            nc.sync.dma_start(out=bias_sb[:, :], in_=bias_d[:, :])
            negtau = consts.tile([P, 1], F32)
            nc.sync.dma_start(out=negtau[:, :], in_=negtau_d[:, :])

            ones64 = consts.tile([D, 1], F32)
            nc.vector.memset(ones64[:, :], 1.0)
            ones64_2 = consts.tile([D, 2], F32)
            nc.vector.memset(ones64_2[:, :], 1.0)
            ones64r = consts.tile([D, 2], F32R)
            nc.vector.tensor_copy(ones64r[:, :], ones64_2[:, :])

            # ---- x arrives pre-transposed/rolled/hi-lo-stacked: [128, n] ----
            xhl = mlp_pool.tile([2 * D, N], F32R, tag="hl_a")
            nc.sync.dma_start(out=xhl[:, :], in_=xhl_d[:, :])

            # ---- 3-layer MLP via f32r hi/lo sweeps (keys only) ----
            # Intermediate hi/lo stay on lanes 0..63 -- both Whi and Wlo
            # exist at both lane ranges via wd/wc slices, so no partition
            # moves are ever needed.
            rhs65 = mlp_pool.tile([D + 1, N], F32, tag="rhs65")
            hmid = mlp_pool.tile([D, N], F32, tag="hmid")
            lo64 = mlp_pool.tile([D, N], F32R, tag="lo64")
            hl_b = mlp_pool.tile([2 * D, N], F32R, tag="hl_b")

            with tc.tile_pool(name="ps_mlp", bufs=1, space="PSUM") as ps_mlp_pool:
                cur = xhl
                for layer in range(3):
                    last = layer == 2
                    pss = []
                    for stat in (wd[layer], wc[layer]):
                        for ci, c in enumerate(range(0, N, MM_F)):
                            if stat is wd[layer]:
                                ps = ps_mlp_pool.tile([D, MM_F], F32, tag=f"m{ci}")
                                pss.append(ps)
                            nc.tensor.matmul(
                                pss[ci][:, :],
                                lhsT=stat[:, :],
                                rhs=cur[:, c:c + MM_F],
                                start=stat is wd[layer],
                                stop=stat is wc[layer],
                            )
                    dst = rhs65 if last else hmid
                    nxt_hl = hl_b if layer == 0 else xhl
                    # chunk-wise drain + hi/lo re-split so the next layer's
                    # matmuls start while later chunks are still splitting
                    for ci, c in enumerate(range(0, N, MM_F)):
                        sl = slice(c, c + MM_F)
                        nc.scalar.activation(
                            out=dst[0:D, sl],
                            in_=pss[ci][:, :],
                            func=AF.Identity if last else AF.Relu,
                            bias=bias_sb[:, (2 if last else layer):
                                         (3 if last else layer + 1)],
                            scale=1.0,
                        )
                        if not last:
                            nc.scalar.activation(
                                out=nxt_hl[0:D, sl], in_=hmid[:, sl], func=AF.Copy
                            )
                            nc.vector.tensor_sub(
                                lo64[:, sl], hmid[:, sl], nxt_hl[0:D, sl]
                            )
                            nc.sync.dma_start(
                                out=nxt_hl[D:2 * D, sl], in_=lo64[:, sl]
                            )
                    if not last:
                        cur = nxt_hl

            # ---- squared norms (keys; queries are a prefix slice) ----
            # sq = ones @ (htsq_hi + htsq_lo): exact f32r split of h*h, both
            # sweeps share the ones stationary (no weight-reload penalty)
            htsq = mlp_pool.tile([D, N], F32, tag="hmid")     # reuse
            nc.vector.tensor_mul(htsq[:, :], rhs65[0:D, :], rhs65[0:D, :])
            htsq_hi = mlp_pool.tile([D, N], F32R, tag="hl_b")  # reuse
            nc.scalar.activation(out=htsq_hi[:, :], in_=htsq[:, :], func=AF.Copy)
            htsq_lo = mlp_pool.tile([D, N], F32R, tag="lo64")  # reuse
            nc.vector.tensor_sub(htsq_lo[:, :], htsq[:, :], htsq_hi[:, :])

            sqrow0 = mlp_pool.tile([1, N], F32, tag="hmid")  # htsq fully read
            with tc.tile_pool(name="ps_sq", bufs=1, space="PSUM") as ps_sq_pool:
                pss = []
                for sweep, src in ((0, htsq_hi), (1, htsq_lo)):
                    for ci, c in enumerate(range(0, N, MM_F)):
                        if sweep == 0:
                            ps = ps_sq_pool.tile([2, MM_F], F32, tag=f"sq{ci}")
                            pss.append(ps)
                        nc.tensor.matmul(
                            pss[ci][0:2, :],
                            lhsT=ones64r[:, :],
                            rhs=src[:, c:c + MM_F],
                            start=sweep == 0, stop=sweep == 1,
                        )
                for ci, c in enumerate(range(0, N, MM_F)):
                    nc.scalar.activation(
                        out=sqrow0[0:1, c:c + MM_F],
                        in_=pss[ci][0:1, :],
                        func=AF.Copy,
                    )

            # ---- f32r hi/lo split of h for the fast distance matmuls ----
            # f32r keeps 11 mantissa bits; products of f32r inputs are exact
            # and accumulate in fp32 PSUM, so
            #   G = hi@hi + lo@lo (MM1, diagonal) + lo@hi + hi@lo (MM2, cross)
            # is fp32-class accurate at ~1 cycle/row instead of 8.
            # rhs_b rows: [hi_k (0:64); lo_k (64:128)]  (lo moved by DMA)
            rhs_b = mlp_pool.tile([P, N], F32R, tag="rhs_b")
            nc.vector.tensor_copy(rhs_b[0:D, :], rhs65[0:D, :])
            lo_tmp = mlp_pool.tile([D, N], F32R, tag="lo64")  # reuse
            nc.vector.tensor_sub(lo_tmp[:, :], rhs65[0:D, :], rhs_b[0:D, :])
            nc.sync.dma_start(out=rhs_b[D:P, :], in_=lo_tmp[:, :])

            # sq2 rows: [sq_hi; sq_lo] (all on lane 0; only sq_lo needs a DMA)
            sq2 = mlp_pool.tile([2, N], F32R, tag="sq2")
            nc.scalar.activation(out=sq2[0:1, :], in_=sqrow0[0:1, :], func=AF.Copy)
            sqlo = mlp_pool.tile([1, N], F32R, tag="lo64")
            nc.vector.tensor_sub(sqlo[0:1, :], sqrow0[0:1, :], sq2[0:1, :])
            nc.sync.dma_start(out=sq2[1:2, :], in_=sqlo[0:1, :])

            # query-side stationary tiles (queries = key cols [0:NQ]):
            # lhs_1 rows [-2hi_q; -2lo_q], lhs_2 rows [-2lo_q; -2hi_q]
            lhs_1 = mlp_pool.tile([P, NQ], F32R, tag="lhs_1")
            lhs_2 = mlp_pool.tile([P, NQ], F32R, tag="lhs_2")
            nc.scalar.activation(
                out=lhs_1[0:D, :], in_=rhs_b[0:D, 0:NQ],
                func=AF.Identity, bias=0.0, scale=-2.0,
            )
            nc.scalar.activation(
                out=lhs_2[0:D, :], in_=lo_tmp[:, 0:NQ],
                func=AF.Identity, bias=0.0, scale=-2.0,
            )
            nc.sync.dma_start(out=lhs_1[D:P, :], in_=lhs_2[0:D, :])
            nc.sync.dma_start(out=lhs_2[D:P, :], in_=lhs_1[0:D, :])

            ones2_f = consts.tile([2, P], F32)
            nc.vector.memset(ones2_f[:, :], 1.0)
            ones2 = consts.tile([2, P], F32R)
            nc.vector.tensor_copy(ones2[:, :], ones2_f[:, :])

            # per-query bias: -tau * sq_q where sq_q = sq_keys[0:NQ].
            # SBUF->SBUF partition scatter can't be balanced in one DMA, so
            # bounce through a small DRAM scratch tensor.
            sqq_dram = nc.dram_tensor("sqq_scratch", [NQ], F32, kind="Internal")
            nc.sync.dma_start(out=sqq_dram[:], in_=sqrow0[0:1, 0:NQ])
            sqt = consts.tile([P, NTQ], F32)
            nc.sync.dma_start(
                out=sqt[:, :],
                in_=sqq_dram[:].rearrange("(t p) -> p t", p=P),
            )
            bias_all = consts.tile([P, NTQ], F32)
            nc.vector.tensor_scalar_mul(bias_all[:, :], sqt[:, :], negtau[:, 0:1])

            # ---- distance blocks + top-8 ----
            vals_sb = out_pool.tile([P, NTQ * K], F32)
            idx_sb = out_pool.tile([P, NTQ * K], U32)

            with tc.tile_pool(name="ps_dist", bufs=2, space="PSUM") as ps_dist_pool:
                for t in range(NTQ):
                    sc = sc_pool.tile([P, N], F32, tag="sc")
                    for h in range(2):
                        ph = ps_dist_pool.tile([P, N // 2], F32, tag="dist")
                        # sweep each stationary over all 4 chunks before
                        # switching: weight reloads cost ~450ns, sweeps ~250ns
                        for phase, (lhsT, rhs) in enumerate((
                            (lhs_1[:, t * P:(t + 1) * P], rhs_b),
                            (lhs_2[:, t * P:(t + 1) * P], rhs_b),
                            (ones2[:, :], sq2),
                        )):
                            for i in range(4):
                                off = h * (N // 2) + i * MM_F
                                nc.tensor.matmul(
                                    ph[:, i * MM_F:(i + 1) * MM_F],
                                    lhsT=lhsT,
                                    rhs=rhs[:, off:off + MM_F],
                                    start=phase == 0, stop=phase == 2,
                                )
                        # scaled = -tau * (sq_j - 2G) - tau*sq_q
                        nc.scalar.activation(
                            out=sc[:, h * (N // 2):(h + 1) * (N // 2)],
                            in_=ph[:, :],
                            func=AF.Identity,
                            bias=bias_all[:, t:t + 1],
                            scale=negtau[:, 0:1],
                        )
                    nc.vector.max(
                        out=vals_sb[:, t * K:(t + 1) * K], in_=sc[:, :]
                    )
                    nc.vector.max_index(
                        out=idx_sb[:, t * K:(t + 1) * K],
                        in_max=vals_sb[:, t * K:(t + 1) * K],
                        in_values=sc[:, :],
                    )

            nc.sync.dma_start(
                out=val_out[:, :].rearrange("(t p) k -> p t k", p=P),
                in_=vals_sb[:, :].rearrange("p (t k) -> p t k", k=K),
            )
            nc.sync.dma_start(
                out=idx_out[:, :].rearrange("(t p) k -> p t k", p=P),
                in_=idx_sb[:, :].rearrange("p (t k) -> p t k", k=K),
            )

    return nc


_NO_HOIST = {"InstEventSemaphore"}


def _legalize_matmul_waits(nc, max_waits=1):
    """Walrus codegen rejects engine instructions with >1 embedded sync wait
    ("Too many sync wait commands"). Hoist excess waits onto standalone
    InstEventSemaphore instructions on the same engine just before the
    instruction (engines execute their stream in order, so semantics are
    preserved)."""
    n_fix = 0
    if not hasattr(nc, "_legalize_dummy_sems"):
        # pick sem numbers that no instruction in the program references
        used = set()
        for f in nc.m.functions:
            for blk in f.blocks:
                for inst in blk.instructions:
                    si = inst.sync_info
                    if si is not None:
                        used.update(w.id for w in si.on_wait)
                        used.update(u.id for u in si.on_update)
        engs = (
            mybir.EngineType.PE,
            mybir.EngineType.Activation,
            mybir.EngineType.DVE,
            mybir.EngineType.Pool,
            mybir.EngineType.SP,
        )
        free = [n for n in range(190, 100, -1) if n not in used]
        nc._legalize_dummy_sems = {
            eng: type("S", (), {"num": free[i], "name": f"lg_dummy_{eng.name}"})()
            for i, eng in enumerate(engs)
        }
    for f in nc.m.functions:
        for blk in f.blocks:
            out = []
            for inst in blk.instructions:
                si = inst.sync_info
                if (
                    type(inst).__name__ not in _NO_HOIST
                    and si is not None
                    and len(si.on_wait) > max_waits
                ):
                    waits = list(si.on_wait)
                    keep, hoist = waits[-max_waits:], waits[:-max_waits]
                    dummy = nc._legalize_dummy_sems[inst.engine]
                    for j, w in enumerate(hoist):
                        upd = mybir.SyncUpdate(
                            sync_type="semaphore",
                            id=dummy.num,
                            ant_name=dummy.name,
                            update_mode="sem-inc",
                            update_value=1,
                        )
                        ev = mybir.InstEventSemaphore(
                            name=f"EVW-{inst.name}-{j}",
                            engine=inst.engine,
                            ins=[],
                            outs=[],
                            sync_info=mybir.SyncInfo(on_wait=[w], on_update=[upd]),
                        )
                        out.append(ev)
                    inst.sync_info = mybir.SyncInfo(
                        on_wait=keep, on_update=list(si.on_update)
                    )
                    n_fix += 1
                out.append(inst)
            blk.instructions[:] = out
    return n_fix


def get_program():
    if "nc" not in _prog_cache:
        nc = _build_program()
        _legalize_matmul_waits(nc)
        _prog_cache["nc"] = nc
    return _prog_cache["nc"]


def make_in_maps(x, W1, b1, W2, b2, W3, b3, temperature):
    x = np.ascontiguousarray(np.asarray(x, dtype=np.float32))
    tau = np.exp(np.clip(np.float32(temperature), np.float32(-5.0),
                         np.float32(5.0)), dtype=np.float32)
    negtau = np.full((P, 1), -tau, dtype=np.float32)
    bias = np.stack(
        [np.asarray(b1, np.float32), np.asarray(b2, np.float32),
         np.asarray(b3, np.float32),
         np.float32(-2.0) * np.asarray(b3, np.float32)],
        axis=1,
    )
    bias = np.ascontiguousarray(bias)

    def trunc11(a):
        """Exact f32r split: hi keeps 11 explicit mantissa bits."""
        b = a.astype(np.float32).view(np.uint32)
        return (b & np.uint32(0xFFFFF000)).view(np.float32)

    def hl_stack(a, rev=False):
        hi = trunc11(a)
        lo = (a - hi).astype(np.float32)
        pair = (lo, hi) if rev else (hi, lo)
        return np.ascontiguousarray(np.concatenate(pair, axis=0))

    wts = {}
    for i, W in enumerate((W1, W2, W3)):
        wt = np.asarray(W, np.float32).T
        wts[f"wd{i}"] = hl_stack(wt)
        wts[f"wc{i}"] = hl_stack(wt, rev=True)

    in_maps = []
    xt_all = [np.ascontiguousarray(x[b_i].T) for b_i in range(B)]
    for c in range(NCORES):
        b_i, half = c // 2, c % 2
        xbt = xt_all[b_i]
        if half:
            # roll keys so this core's queries are key columns [0:NQ];
            # local key j holds global key (j + NQ) % N
            xbt = np.ascontiguousarray(np.roll(xbt, -NQ, axis=1))
        in_maps.append({
            "xhl": hl_stack(xbt),
            "bias": bias,
            "negtau": negtau,
            **wts,
        })
    return in_maps


def assemble_outputs(results):
    """results: list of 8 dicts with val_out [NQ,K] f32, idx_out [NQ,K] u32."""
    edges = np.empty((B, N * K, 2), dtype=np.int32)
    logprobs = np.empty((B, N, K), dtype=np.float32)
    rows = np.repeat(np.arange(N, dtype=np.int32), K)
    for b_i in range(B):
        idx = np.concatenate(
            [results[2 * b_i]["idx_out"].astype(np.int32),
             # second half-core saw keys rolled by NQ: un-roll indices
             (results[2 * b_i + 1]["idx_out"].astype(np.int32) + NQ) % N],
            axis=0,
        )
        vals = np.concatenate(
            [results[2 * b_i]["val_out"], results[2 * b_i + 1]["val_out"]], axis=0
        )
        edges[b_i, :, 0] = idx.reshape(-1)
        edges[b_i, :, 1] = rows
        logprobs[b_i] = vals
    return edges, logprobs


def run(inputs, trace=False):
    """Full pipeline; returns ((edges, logprobs), BassKernelResults)."""
    k = int(np.asarray(inputs["k"]))
    assert k == K, f"kernel hardcodes k=8, got {k}"
    nc = get_program()
    in_maps = make_in_maps(
        inputs["x"], inputs["W1"], inputs["b1"], inputs["W2"], inputs["b2"],
        inputs["W3"], inputs["b3"], inputs["temperature"],
    )
    br = run_bass_kernel_spmd(nc, in_maps, list(range(NCORES)), trace=trace)
    return assemble_outputs(br.results), br


def kernel(**inputs):
    (edges, logprobs), _ = run(inputs, trace=False)
    return edges, logprobs


# revision 75
# speedup vs baseline: 1.0514x; 1.0450x over previous
"""DGM kNN kernel for Trainium2 (Bass/Tile), SPMD over 8 NeuronCores.

Problem: 3-layer MLP on x[4,4096,64], pairwise sq-distances per batch,
top-k=8 smallest per row (scaled by tau), outputs (edges, logprobs).

Sharding: core c handles batch c//2, query rows (c%2)*2048 ..+2048.
Each core computes the full-batch MLP (keys) + its query half, the
[2048, 4096] scaled-distance block, and native max8/max_index top-8.
"""

import numpy as np

import concourse.bass as bass
import concourse.mybir as mybir
import concourse.tile as tile
from concourse.bass_utils import run_bass_kernel_spmd

F32 = mybir.dt.float32
F32R = mybir.dt.float32r
U32 = mybir.dt.uint32
AF = mybir.ActivationFunctionType

B, N, D, K = 4, 4096, 64, 8
NCORES = 8
NQ = N // 2          # query rows per core
P = 128
NT = N // P          # 32 x-tiles (keys)
NTQ = NQ // P        # 16 x-tiles (queries) == q tiles
MM_F = 512           # matmul moving free dim (one PSUM bank of fp32)

_prog_cache = {}


def _build_program():
    nc = bass.Bass("TRN2")
    # reserved before TileContext so the numbers never collide with tile sems;
    # used by _legalize_matmul_waits as no-op update targets (one per engine
    # so the race detector sees a single updater per sem)
    nc._legalize_dummy_sems = {
        eng: nc.alloc_semaphore(name=f"legalize_dummy_{eng.name}")
        for eng in (
            mybir.EngineType.PE,
            mybir.EngineType.Activation,
            mybir.EngineType.DVE,
            mybir.EngineType.Pool,
            mybir.EngineType.SP,
        )
    }

    # keys arrive pre-transposed AND rolled per core so that this core's
    # query block is always key columns [0:NQ] (host un-rolls indices).
    # x and W come pre-split into f32r hi/lo pairs (exact decomposition),
    # stacked along the contract dim: [hi(64); lo(64)].
    xhl_d = nc.dram_tensor("xhl", [2 * D, N], F32R, kind="ExternalInput")
    # per layer: diag stack [Whi; Wlo] and cross stack [Wlo; Whi]
    wd_d = [nc.dram_tensor(f"wd{i}", [2 * D, D], F32R, kind="ExternalInput")
            for i in range(3)]
    wc_d = [nc.dram_tensor(f"wc{i}", [2 * D, D], F32R, kind="ExternalInput")
            for i in range(3)]
    # bias columns: b1, b2, b3, -2*b3
    bias_d = nc.dram_tensor("bias", [D, 4], F32, kind="ExternalInput")
    # -tau broadcast per partition
    negtau_d = nc.dram_tensor("negtau", [P, 1], F32, kind="ExternalInput")

    val_out = nc.dram_tensor("val_out", [NQ, K], F32, kind="ExternalOutput")
    idx_out = nc.dram_tensor("idx_out", [NQ, K], U32, kind="ExternalOutput")

    with tile.TileContext(nc) as tc:
        with (
            tc.tile_pool(name="consts", bufs=1) as consts,
            tc.tile_pool(name="mlp", bufs=1) as mlp_pool,
            tc.tile_pool(name="scaled", bufs=3) as sc_pool,
            tc.tile_pool(name="outs", bufs=1) as out_pool,
        ):
            wd, wc = [], []
            for i in range(3):
                w1 = consts.tile([2 * D, D], F32R, tag=f"wd{i}")
                nc.sync.dma_start(out=w1[:, :], in_=wd_d[i][:, :])
                wd.append(w1)
                w2 = consts.tile([2 * D, D], F32R, tag=f"wc{i}")
                nc.sync.dma_start(out=w2[:, :], in_=wc_d[i][:, :])
                wc.append(w2)
            bias_sb = consts.tile([D, 4], F32)
            nc.sync.dma_start(out=bias_sb[:, :], in_=bias_d[:, :])
            negtau = consts.tile([P, 1], F32)
            nc.sync.dma_start(out=negtau[:, :], in_=negtau_d[:, :])

            ones64 = consts.tile([D, 1], F32)
            nc.vector.memset(ones64[:, :], 1.0)
            ones64_2 = consts.tile([D, 2], F32)
            nc.vector.memset(ones64_2[:, :], 1.0)
            ones64r = consts.tile([D, 2], F32R)
            nc.vector.tensor_copy(ones64r[:, :], ones64_2[:, :])

            # ---- x arrives pre-transposed/rolled/hi-lo-stacked: [128, n] ----
            # chunked so layer-1's first sweep starts after 1/8 of the load
            xhl = mlp_pool.tile([2 * D, N], F32R, tag="hl_a")
            for c in range(0, N, MM_F):
                nc.sync.dma_start(
                    out=xhl[:, c:c + MM_F], in_=xhl_d[:, c:c + MM_F]
                )

            # ---- 3-layer MLP via f32r hi/lo sweeps (keys only) ----
            # Intermediate hi/lo stay on lanes 0..63 -- both Whi and Wlo
            # exist at both lane ranges via wd/wc slices, so no partition
            # moves are ever needed.
            rhs65 = mlp_pool.tile([D + 1, N], F32, tag="rhs65")
            hmid = mlp_pool.tile([D, N], F32, tag="hmid")
            lo64 = mlp_pool.tile([D, N], F32R, tag="lo64")
            hl_b = mlp_pool.tile([2 * D, N], F32R, tag="hl_b")

            with tc.tile_pool(name="ps_mlp", bufs=1, space="PSUM") as ps_mlp_pool:
                cur = xhl
                for layer in range(3):
                    last = layer == 2
                    pss = []
                    for stat in (wd[layer], wc[layer]):
                        for ci, c in enumerate(range(0, N, MM_F)):
                            if stat is wd[layer]:
                                ps = ps_mlp_pool.tile([D, MM_F], F32, tag=f"m{ci}")
                                pss.append(ps)
                            nc.tensor.matmul(
                                pss[ci][:, :],
                                lhsT=stat[:, :],
                                rhs=cur[:, c:c + MM_F],
                                start=stat is wd[layer],
                                stop=stat is wc[layer],
                            )
                    dst = rhs65 if last else hmid
                    nxt_hl = hl_b if layer == 0 else xhl
                    # chunk-wise drain + hi/lo re-split so the next layer's
                    # matmuls start while later chunks are still splitting
                    for ci, c in enumerate(range(0, N, MM_F)):
                        sl = slice(c, c + MM_F)
                        nc.scalar.activation(
                            out=dst[0:D, sl],
                            in_=pss[ci][:, :],
                            func=AF.Identity if last else AF.Relu,
                            bias=bias_sb[:, (2 if last else layer):
                                         (3 if last else layer + 1)],
                            scale=1.0,
                        )
                        if not last:
                            nc.scalar.activation(
                                out=nxt_hl[0:D, sl], in_=hmid[:, sl], func=AF.Copy
                            )
                            nc.vector.tensor_sub(
                                lo64[:, sl], hmid[:, sl], nxt_hl[0:D, sl]
                            )
                            nc.sync.dma_start(
                                out=nxt_hl[D:2 * D, sl], in_=lo64[:, sl]
                            )
                    if not last:
                        cur = nxt_hl

            # ---- squared norms (keys; queries are a prefix slice) ----
            # sq = ones @ (htsq_hi + htsq_lo): exact f32r split of h*h, both
            # sweeps share the ones stationary (no weight-reload penalty)
            htsq = mlp_pool.tile([D, N], F32, tag="hmid")     # reuse
            nc.vector.tensor_mul(htsq[:, :], rhs65[0:D, :], rhs65[0:D, :])
            htsq_hi = mlp_pool.tile([D, N], F32R, tag="hl_b")  # reuse
            nc.scalar.activation(out=htsq_hi[:, :], in_=htsq[:, :], func=AF.Copy)
            htsq_lo = mlp_pool.tile([D, N], F32R, tag="lo64")  # reuse
            nc.vector.tensor_sub(htsq_lo[:, :], htsq[:, :], htsq_hi[:, :])

            sqrow0 = mlp_pool.tile([1, N], F32, tag="hmid")  # htsq fully read
            with tc.tile_pool(name="ps_sq", bufs=1, space="PSUM") as ps_sq_pool:
                pss = []
                for sweep, src in ((0, htsq_hi), (1, htsq_lo)):
                    for ci, c in enumerate(range(0, N, MM_F)):
                        if sweep == 0:
                            ps = ps_sq_pool.tile([2, MM_F], F32, tag=f"sq{ci}")
                            pss.append(ps)
                        nc.tensor.matmul(
                            pss[ci][0:2, :],
                            lhsT=ones64r[:, :],
                            rhs=src[:, c:c + MM_F],
                            start=sweep == 0, stop=sweep == 1,
                        )
                for ci, c in enumerate(range(0, N, MM_F)):
                    nc.scalar.activation(
                        out=sqrow0[0:1, c:c + MM_F],
                        in_=pss[ci][0:1, :],
                        func=AF.Copy,
                    )

            # ---- f32r hi/lo split of h for the fast distance matmuls ----
            # f32r keeps 11 mantissa bits; products of f32r inputs are exact
            # and accumulate in fp32 PSUM, so
            #   G = hi@hi + lo@lo (MM1, diagonal) + lo@hi + hi@lo (MM2, cross)
            # is fp32-class accurate at ~1 cycle/row instead of 8.
            # rhs_b rows: [hi_k (0:64); lo_k (64:128)]  (lo moved by DMA)
            rhs_b = mlp_pool.tile([P, N], F32R, tag="rhs_b")
            nc.vector.tensor_copy(rhs_b[0:D, :], rhs65[0:D, :])
            lo_tmp = mlp_pool.tile([D, N], F32R, tag="lo64")  # reuse
            nc.vector.tensor_sub(lo_tmp[:, :], rhs65[0:D, :], rhs_b[0:D, :])
            nc.sync.dma_start(out=rhs_b[D:P, :], in_=lo_tmp[:, :])

            # sq2 rows: [sq_hi; sq_lo] (all on lane 0; only sq_lo needs a DMA)
            sq2 = mlp_pool.tile([2, N], F32R, tag="sq2")
            nc.scalar.activation(out=sq2[0:1, :], in_=sqrow0[0:1, :], func=AF.Copy)
            sqlo = mlp_pool.tile([1, N], F32R, tag="lo64")
            nc.vector.tensor_sub(sqlo[0:1, :], sqrow0[0:1, :], sq2[0:1, :])
            nc.sync.dma_start(out=sq2[1:2, :], in_=sqlo[0:1, :])

            # query-side stationary tiles (queries = key cols [0:NQ]):
            # lhs_1 rows [-2hi_q; -2lo_q], lhs_2 rows [-2lo_q; -2hi_q]
            lhs_1 = mlp_pool.tile([P, NQ], F32R, tag="lhs_1")
            lhs_2 = mlp_pool.tile([P, NQ], F32R, tag="lhs_2")
            nc.scalar.activation(
                out=lhs_1[0:D, :], in_=rhs_b[0:D, 0:NQ],
                func=AF.Identity, bias=0.0, scale=-2.0,
            )
            nc.scalar.activation(
                out=lhs_2[0:D, :], in_=lo_tmp[:, 0:NQ],
                func=AF.Identity, bias=0.0, scale=-2.0,
            )
            nc.sync.dma_start(out=lhs_1[D:P, :], in_=lhs_2[0:D, :])
            nc.sync.dma_start(out=lhs_2[D:P, :], in_=lhs_1[0:D, :])

            ones2_f = consts.tile([2, P], F32)
            nc.vector.memset(ones2_f[:, :], 1.0)
            ones2 = consts.tile([2, P], F32R)
            nc.vector.tensor_copy(ones2[:, :], ones2_f[:, :])

            # per-query bias: -tau * sq_q where sq_q = sq_keys[0:NQ].
            # SBUF->SBUF partition scatter can't be balanced in one DMA, so
            # bounce through a small DRAM scratch tensor.
            sqq_dram = nc.dram_tensor("sqq_scratch", [NQ], F32, kind="Internal")
            nc.sync.dma_start(out=sqq_dram[:], in_=sqrow0[0:1, 0:NQ])
            sqt = consts.tile([P, NTQ], F32)
            nc.sync.dma_start(
                out=sqt[:, :],
                in_=sqq_dram[:].rearrange("(t p) -> p t", p=P),
            )
            bias_all = consts.tile([P, NTQ], F32)
            nc.vector.tensor_scalar_mul(bias_all[:, :], sqt[:, :], negtau[:, 0:1])

            # ---- distance blocks + top-8 ----
            vals_sb = out_pool.tile([P, NTQ * K], F32)
            idx_sb = out_pool.tile([P, NTQ * K], U32)

            with tc.tile_pool(name="ps_dist", bufs=2, space="PSUM") as ps_dist_pool:
                for t in range(NTQ):
                    sc = sc_pool.tile([P, N], F32, tag="sc")
                    for h in range(2):
                        ph = ps_dist_pool.tile([P, N // 2], F32, tag="dist")
                        # sweep each stationary over all 4 chunks before
                        # switching: weight reloads cost ~450ns, sweeps ~250ns
                        for phase, (lhsT, rhs) in enumerate((
                            (lhs_1[:, t * P:(t + 1) * P], rhs_b),
                            (lhs_2[:, t * P:(t + 1) * P], rhs_b),
                            (ones2[:, :], sq2),
                        )):
                            for i in range(4):
                                off = h * (N // 2) + i * MM_F
                                nc.tensor.matmul(
                                    ph[:, i * MM_F:(i + 1) * MM_F],
                                    lhsT=lhsT,
                                    rhs=rhs[:, off:off + MM_F],
                                    start=phase == 0, stop=phase == 2,
                                )
                        # scaled = -tau * (sq_j - 2G) - tau*sq_q
                        nc.scalar.activation(
                            out=sc[:, h * (N // 2):(h + 1) * (N // 2)],
                            in_=ph[:, :],
                            func=AF.Identity,
                            bias=bias_all[:, t:t + 1],
                            scale=negtau[:, 0:1],
                        )
                    nc.vector.max(
                        out=vals_sb[:, t * K:(t + 1) * K], in_=sc[:, :]
                    )
                    nc.vector.max_index(
                        out=idx_sb[:, t * K:(t + 1) * K],
                        in_max=vals_sb[:, t * K:(t + 1) * K],
                        in_values=sc[:, :],
                    )

            nc.sync.dma_start(
                out=val_out[:, :].rearrange("(t p) k -> p t k", p=P),
                in_=vals_sb[:, :].rearrange("p (t k) -> p t k", k=K),
            )
            nc.sync.dma_start(
                out=idx_out[:, :].rearrange("(t p) k -> p t k", p=P),
                in_=idx_sb[:, :].rearrange("p (t k) -> p t k", k=K),
            )

    return nc


_NO_HOIST = {"InstEventSemaphore"}


def _legalize_matmul_waits(nc, max_waits=1):
    """Walrus codegen rejects engine instructions with >1 embedded sync wait
    ("Too many sync wait commands"). Hoist excess waits onto standalone
    InstEventSemaphore instructions on the same engine just before the
    instruction (engines execute their stream in order, so semantics are
    preserved)."""
    n_fix = 0
    if not hasattr(nc, "_legalize_dummy_sems"):
        # pick sem numbers that no instruction in the program references
        used = set()
        for f in nc.m.functions:
            for blk in f.blocks:
                for inst in blk.instructions:
                    si = inst.sync_info
                    if si is not None:
                        used.update(w.id for w in si.on_wait)
                        used.update(u.id for u in si.on_update)
        engs = (
            mybir.EngineType.PE,
            mybir.EngineType.Activation,
            mybir.EngineType.DVE,
            mybir.EngineType.Pool,
            mybir.EngineType.SP,
        )
        free = [n for n in range(190, 100, -1) if n not in used]
        nc._legalize_dummy_sems = {
            eng: type("S", (), {"num": free[i], "name": f"lg_dummy_{eng.name}"})()
            for i, eng in enumerate(engs)
        }
    for f in nc.m.functions:
        for blk in f.blocks:
            out = []
            for inst in blk.instructions:
                si = inst.sync_info
                if (
                    type(inst).__name__ not in _NO_HOIST
                    and si is not None
                    and len(si.on_wait) > max_waits
                ):
                    waits = list(si.on_wait)
                    keep, hoist = waits[-max_waits:], waits[:-max_waits]
                    dummy = nc._legalize_dummy_sems[inst.engine]
                    for j, w in enumerate(hoist):
                        upd = mybir.SyncUpdate(
                            sync_type="semaphore",
                            id=dummy.num,
                            ant_name=dummy.name,
                            update_mode="sem-inc",
                            update_value=1,
                        )
                        ev = mybir.InstEventSemaphore(
                            name=f"EVW-{inst.name}-{j}",
                            engine=inst.engine,
                            ins=[],
                            outs=[],
                            sync_info=mybir.SyncInfo(on_wait=[w], on_update=[upd]),
                        )
                        out.append(ev)
                    inst.sync_info = mybir.SyncInfo(
                        on_wait=keep, on_update=list(si.on_update)
                    )
                    n_fix += 1
                out.append(inst)
            blk.instructions[:] = out
    return n_fix


def get_program():
    if "nc" not in _prog_cache:
        nc = _build_program()
        _legalize_matmul_waits(nc)
        _prog_cache["nc"] = nc
    return _prog_cache["nc"]


def make_in_maps(x, W1, b1, W2, b2, W3, b3, temperature):
    x = np.ascontiguousarray(np.asarray(x, dtype=np.float32))
    tau = np.exp(np.clip(np.float32(temperature), np.float32(-5.0),
                         np.float32(5.0)), dtype=np.float32)
    negtau = np.full((P, 1), -tau, dtype=np.float32)
    bias = np.stack(
        [np.asarray(b1, np.float32), np.asarray(b2, np.float32),
         np.asarray(b3, np.float32),
         np.float32(-2.0) * np.asarray(b3, np.float32)],
        axis=1,
    )
    bias = np.ascontiguousarray(bias)

    def trunc11(a):
        """Exact f32r split: hi keeps 11 explicit mantissa bits."""
        b = a.astype(np.float32).view(np.uint32)
        return (b & np.uint32(0xFFFFF000)).view(np.float32)

    def hl_stack(a, rev=False):
        hi = trunc11(a)
        lo = (a - hi).astype(np.float32)
        pair = (lo, hi) if rev else (hi, lo)
        return np.ascontiguousarray(np.concatenate(pair, axis=0))

    wts = {}
    for i, W in enumerate((W1, W2, W3)):
        wt = np.asarray(W, np.float32).T
        wts[f"wd{i}"] = hl_stack(wt)
        wts[f"wc{i}"] = hl_stack(wt, rev=True)

    in_maps = []
    xt_all = [np.ascontiguousarray(x[b_i].T) for b_i in range(B)]
    for c in range(NCORES):
        b_i, half = c // 2, c % 2
        xbt = xt_all[b_i]
        if half:
            # roll keys so this core's queries are key columns [0:NQ];
            # local key j holds global key (j + NQ) % N
            xbt = np.ascontiguousarray(np.roll(xbt, -NQ, axis=1))
        in_maps.append({
            "xhl": hl_stack(xbt),
            "bias": bias,
            "negtau": negtau,
            **wts,
        })
    return in_maps


def assemble_outputs(results):
    """results: list of 8 dicts with val_out [NQ,K] f32, idx_out [NQ,K] u32."""
    edges = np.empty((B, N * K, 2), dtype=np.int32)
    logprobs = np.empty((B, N, K), dtype=np.float32)
    rows = np.repeat(np.arange(N, dtype=np.int32), K)
    for b_i in range(B):
        idx = np.concatenate(
            [results[2 * b_i]["idx_out"].astype(np.int32),
             # second half-core saw keys rolled by NQ: un-roll indices
             (results[2 * b_i + 1]["idx_out"].astype(np.int32) + NQ) % N],
            axis=0,
        )
        vals = np.concatenate(
            [results[2 * b_i]["val_out"], results[2 * b_i + 1]["val_out"]], axis=0
        )
        edges[b_i, :, 0] = idx.reshape(-1)
        edges[b_i, :, 1] = rows
        logprobs[b_i] = vals
    return edges, logprobs


def run(inputs, trace=False):
    """Full pipeline; returns ((edges, logprobs), BassKernelResults)."""
    k = int(np.asarray(inputs["k"]))
    assert k == K, f"kernel hardcodes k=8, got {k}"
    nc = get_program()
    in_maps = make_in_maps(
        inputs["x"], inputs["W1"], inputs["b1"], inputs["W2"], inputs["b2"],
        inputs["W3"], inputs["b3"], inputs["temperature"],
    )
    br = run_bass_kernel_spmd(nc, in_maps, list(range(NCORES)), trace=trace)
    return assemble_outputs(br.results), br


def kernel(**inputs):
    (edges, logprobs), _ = run(inputs, trace=False)
    return edges, logprobs


# revision 78
# speedup vs baseline: 1.0670x; 1.0149x over previous
"""DGM kNN kernel for Trainium2 (Bass/Tile), SPMD over 8 NeuronCores.

Problem: 3-layer MLP on x[4,4096,64], pairwise sq-distances per batch,
top-k=8 smallest per row (scaled by tau), outputs (edges, logprobs).

Sharding: core c handles batch c//2, query rows (c%2)*2048 ..+2048.
Each core computes the full-batch MLP (keys) + its query half, the
[2048, 4096] scaled-distance block, and native max8/max_index top-8.
"""

import numpy as np

import concourse.bass as bass
import concourse.mybir as mybir
import concourse.tile as tile
from concourse.bass_utils import run_bass_kernel_spmd

F32 = mybir.dt.float32
F32R = mybir.dt.float32r
U32 = mybir.dt.uint32
AF = mybir.ActivationFunctionType

B, N, D, K = 4, 4096, 64, 8
NCORES = 8
NQ = N // 2          # query rows per core
P = 128
NT = N // P          # 32 x-tiles (keys)
NTQ = NQ // P        # 16 x-tiles (queries) == q tiles
MM_F = 512           # matmul moving free dim (one PSUM bank of fp32)

_prog_cache = {}


def _build_program():
    nc = bass.Bass("TRN2")
    # reserved before TileContext so the numbers never collide with tile sems;
    # used by _legalize_matmul_waits as no-op update targets (one per engine
    # so the race detector sees a single updater per sem)
    nc._legalize_dummy_sems = {
        eng: nc.alloc_semaphore(name=f"legalize_dummy_{eng.name}")
        for eng in (
            mybir.EngineType.PE,
            mybir.EngineType.Activation,
            mybir.EngineType.DVE,
            mybir.EngineType.Pool,
            mybir.EngineType.SP,
        )
    }

    # keys arrive pre-transposed AND rolled per core so that this core's
    # query block is always key columns [0:NQ] (host un-rolls indices).
    # x and W come pre-split into f32r hi/lo pairs (exact decomposition),
    # stacked along the contract dim: [hi(64); lo(64)].
    xhl_d = nc.dram_tensor("xhl", [2 * D, N], F32R, kind="ExternalInput")
    # per layer: diag stack [Whi; Wlo] and cross stack [Wlo; Whi]
    wd_d = [nc.dram_tensor(f"wd{i}", [2 * D, D], F32R, kind="ExternalInput")
            for i in range(3)]
    wc_d = [nc.dram_tensor(f"wc{i}", [2 * D, D], F32R, kind="ExternalInput")
            for i in range(3)]
    # bias columns: b1, b2, b3, -2*b3
    bias_d = nc.dram_tensor("bias", [D, 4], F32, kind="ExternalInput")
    # -tau broadcast per partition
    negtau_d = nc.dram_tensor("negtau", [P, 1], F32, kind="ExternalInput")

    val_out = nc.dram_tensor("val_out", [NQ, K], F32, kind="ExternalOutput")
    idx_out = nc.dram_tensor("idx_out", [NQ, K], U32, kind="ExternalOutput")

    with tile.TileContext(nc) as tc:
        with (
            tc.tile_pool(name="consts", bufs=1) as consts,
            tc.tile_pool(name="mlp", bufs=1) as mlp_pool,
            tc.tile_pool(name="scaled", bufs=3) as sc_pool,
            tc.tile_pool(name="outs", bufs=1) as out_pool,
        ):
            wd, wc = [], []
            for i in range(3):
                w1 = consts.tile([2 * D, D], F32R, tag=f"wd{i}")
                nc.sync.dma_start(out=w1[:, :], in_=wd_d[i][:, :])
                wd.append(w1)
                w2 = consts.tile([2 * D, D], F32R, tag=f"wc{i}")
                nc.sync.dma_start(out=w2[:, :], in_=wc_d[i][:, :])
                wc.append(w2)
            bias_sb = consts.tile([D, 4], F32)
            nc.sync.dma_start(out=bias_sb[:, :], in_=bias_d[:, :])
            negtau = consts.tile([P, 1], F32)
            nc.sync.dma_start(out=negtau[:, :], in_=negtau_d[:, :])

            ones64 = consts.tile([D, 1], F32)
            nc.vector.memset(ones64[:, :], 1.0)
            ones64_2 = consts.tile([D, 2], F32)
            nc.vector.memset(ones64_2[:, :], 1.0)
            ones64r = consts.tile([D, 2], F32R)
            nc.vector.tensor_copy(ones64r[:, :], ones64_2[:, :])

            # ---- x arrives pre-transposed/rolled/hi-lo-stacked: [128, n] ----
            # chunked so layer-1's first sweep starts after 1/8 of the load
            xhl = mlp_pool.tile([2 * D, N], F32R, tag="hl_a")
            for c in range(0, N, MM_F):
                nc.sync.dma_start(
                    out=xhl[:, c:c + MM_F], in_=xhl_d[:, c:c + MM_F]
                )

            # ---- 3-layer MLP via f32r hi/lo sweeps (keys only) ----
            # Intermediate hi/lo stay on lanes 0..63 -- both Whi and Wlo
            # exist at both lane ranges via wd/wc slices, so no partition
            # moves are ever needed.
            rhs65 = mlp_pool.tile([D + 1, N], F32, tag="rhs65")
            hmid = mlp_pool.tile([D, N], F32, tag="hmid")
            lo64 = mlp_pool.tile([D, N], F32R, tag="lo64")
            hl_b = mlp_pool.tile([2 * D, N], F32R, tag="hl_b")

            with tc.tile_pool(name="ps_mlp", bufs=1, space="PSUM") as ps_mlp_pool:
                cur = xhl
                for layer in range(3):
                    last = layer == 2
                    pss = []
                    for stat in (wd[layer], wc[layer]):
                        for ci, c in enumerate(range(0, N, MM_F)):
                            if stat is wd[layer]:
                                ps = ps_mlp_pool.tile([D, MM_F], F32, tag=f"m{ci}")
                                pss.append(ps)
                            nc.tensor.matmul(
                                pss[ci][:, :],
                                lhsT=stat[:, :],
                                rhs=cur[:, c:c + MM_F],
                                start=stat is wd[layer],
                                stop=stat is wc[layer],
                            )
                    dst = rhs65 if last else hmid
                    nxt_hl = hl_b if layer == 0 else xhl
                    # chunk-wise drain + hi/lo re-split so the next layer's
                    # matmuls start while later chunks are still splitting
                    for ci, c in enumerate(range(0, N, MM_F)):
                        sl = slice(c, c + MM_F)
                        nc.scalar.activation(
                            out=dst[0:D, sl],
                            in_=pss[ci][:, :],
                            func=AF.Identity if last else AF.Relu,
                            bias=bias_sb[:, (2 if last else layer):
                                         (3 if last else layer + 1)],
                            scale=1.0,
                        )
                        if not last:
                            nc.scalar.activation(
                                out=nxt_hl[0:D, sl], in_=hmid[:, sl], func=AF.Copy
                            )
                            nc.vector.tensor_sub(
                                lo64[:, sl], hmid[:, sl], nxt_hl[0:D, sl]
                            )
                            nc.sync.dma_start(
                                out=nxt_hl[D:2 * D, sl], in_=lo64[:, sl]
                            )
                    if not last:
                        cur = nxt_hl

            # ---- squared norms (keys; queries are a prefix slice) ----
            # sq = ones @ (htsq_hi + htsq_lo): exact f32r split of h*h, both
            # sweeps share the ones stationary (no weight-reload penalty)
            htsq = mlp_pool.tile([D, N], F32, tag="hmid")     # reuse
            htsq_hi = mlp_pool.tile([D, N], F32R, tag="hl_b")  # reuse
            htsq_lo = mlp_pool.tile([D, N], F32R, tag="lo64")  # reuse
            for c in range(0, N, MM_F):
                sl = slice(c, c + MM_F)
                nc.vector.tensor_mul(htsq[:, sl], rhs65[0:D, sl], rhs65[0:D, sl])
                nc.scalar.activation(
                    out=htsq_hi[:, sl], in_=htsq[:, sl], func=AF.Copy
                )
                nc.vector.tensor_sub(
                    htsq_lo[:, sl], htsq[:, sl], htsq_hi[:, sl]
                )

            sqrow0 = mlp_pool.tile([1, N], F32, tag="hmid")  # htsq fully read
            with tc.tile_pool(name="ps_sq", bufs=1, space="PSUM") as ps_sq_pool:
                pss = []
                for sweep, src in ((0, htsq_hi), (1, htsq_lo)):
                    for ci, c in enumerate(range(0, N, MM_F)):
                        if sweep == 0:
                            ps = ps_sq_pool.tile([2, MM_F], F32, tag=f"sq{ci}")
                            pss.append(ps)
                        nc.tensor.matmul(
                            pss[ci][0:2, :],
                            lhsT=ones64r[:, :],
                            rhs=src[:, c:c + MM_F],
                            start=sweep == 0, stop=sweep == 1,
                        )
                for ci, c in enumerate(range(0, N, MM_F)):
                    nc.scalar.activation(
                        out=sqrow0[0:1, c:c + MM_F],
                        in_=pss[ci][0:1, :],
                        func=AF.Copy,
                    )

            # ---- f32r hi/lo split of h for the fast distance matmuls ----
            # f32r keeps 11 mantissa bits; products of f32r inputs are exact
            # and accumulate in fp32 PSUM, so
            #   G = hi@hi + lo@lo (MM1, diagonal) + lo@hi + hi@lo (MM2, cross)
            # is fp32-class accurate at ~1 cycle/row instead of 8.
            # rhs_b rows: [hi_k (0:64); lo_k (64:128)]  (lo moved by DMA)
            rhs_b = mlp_pool.tile([P, N], F32R, tag="rhs_b")
            lo_tmp = mlp_pool.tile([D, N], F32R, tag="lo64")  # reuse
            for c in range(0, N, MM_F):
                sl = slice(c, c + MM_F)
                nc.vector.tensor_copy(rhs_b[0:D, sl], rhs65[0:D, sl])
                nc.vector.tensor_sub(
                    lo_tmp[:, sl], rhs65[0:D, sl], rhs_b[0:D, sl]
                )
                nc.sync.dma_start(out=rhs_b[D:P, sl], in_=lo_tmp[:, sl])

            # sq2 rows: [sq_hi; sq_lo] on lane 0 (off the critical path --
            # MM3 is the last phase per half)
            sq2 = mlp_pool.tile([2, N], F32R, tag="sq2")
            nc.scalar.activation(out=sq2[0:1, :], in_=sqrow0[0:1, :], func=AF.Copy)
            sqlo = mlp_pool.tile([1, N], F32R, tag="lo64")
            nc.vector.tensor_sub(sqlo[0:1, :], sqrow0[0:1, :], sq2[0:1, :])
            nc.sync.dma_start(out=sq2[1:2, :], in_=sqlo[0:1, :])

            # query-side stationary tiles (queries = key cols [0:NQ]):
            # lhs_1 rows [-2hi_q; -2lo_q], lhs_2 rows [-2lo_q; -2hi_q]
            lhs_1 = mlp_pool.tile([P, NQ], F32R, tag="lhs_1")
            lhs_2 = mlp_pool.tile([P, NQ], F32R, tag="lhs_2")
            nc.scalar.activation(
                out=lhs_1[0:D, :], in_=rhs_b[0:D, 0:NQ],
                func=AF.Identity, bias=0.0, scale=-2.0,
            )
            nc.scalar.activation(
                out=lhs_2[0:D, :], in_=lo_tmp[:, 0:NQ],
                func=AF.Identity, bias=0.0, scale=-2.0,
            )
            nc.sync.dma_start(out=lhs_1[D:P, :], in_=lhs_2[0:D, :])
            nc.sync.dma_start(out=lhs_2[D:P, :], in_=lhs_1[0:D, :])

            ones2_f = consts.tile([2, P], F32)
            nc.vector.memset(ones2_f[:, :], 1.0)
            ones2 = consts.tile([2, P], F32R)
            nc.vector.tensor_copy(ones2[:, :], ones2_f[:, :])

            # per-query bias: -tau * sq_q where sq_q = sq_keys[0:NQ].
            # SBUF->SBUF partition scatter can't be balanced in one DMA, so
            # bounce through a small DRAM scratch tensor.
            sqq_dram = nc.dram_tensor("sqq_scratch", [NQ], F32, kind="Internal")
            nc.sync.dma_start(out=sqq_dram[:], in_=sqrow0[0:1, 0:NQ])
            sqt = consts.tile([P, NTQ], F32)
            nc.sync.dma_start(
                out=sqt[:, :],
                in_=sqq_dram[:].rearrange("(t p) -> p t", p=P),
            )
            bias_all = consts.tile([P, NTQ], F32)
            nc.vector.tensor_scalar_mul(bias_all[:, :], sqt[:, :], negtau[:, 0:1])

            # ---- distance blocks + top-8 ----
            vals_sb = out_pool.tile([P, NTQ * K], F32)
            idx_sb = out_pool.tile([P, NTQ * K], U32)

            with tc.tile_pool(name="ps_dist", bufs=2, space="PSUM") as ps_dist_pool:
                for t in range(NTQ):
                    sc = sc_pool.tile([P, N], F32, tag="sc")
                    for h in range(2):
                        ph = ps_dist_pool.tile([P, N // 2], F32, tag="dist")
                        # sweep each stationary over all 4 chunks before
                        # switching: weight reloads cost ~450ns, sweeps ~250ns
                        for phase, (lhsT, rhs) in enumerate((
                            (lhs_1[:, t * P:(t + 1) * P], rhs_b),
                            (lhs_2[:, t * P:(t + 1) * P], rhs_b),
                            (ones2[:, :], sq2),
                        )):
                            for i in range(4):
                                off = h * (N // 2) + i * MM_F
                                nc.tensor.matmul(
                                    ph[:, i * MM_F:(i + 1) * MM_F],
                                    lhsT=lhsT,
                                    rhs=rhs[:, off:off + MM_F],
                                    start=phase == 0, stop=phase == 2,
                                )
                        # scaled = -tau * (sq_j - 2G) - tau*sq_q
                        nc.scalar.activation(
                            out=sc[:, h * (N // 2):(h + 1) * (N // 2)],
                            in_=ph[:, :],
                            func=AF.Identity,
                            bias=bias_all[:, t:t + 1],
                            scale=negtau[:, 0:1],
                        )
                    nc.vector.max(
                        out=vals_sb[:, t * K:(t + 1) * K], in_=sc[:, :]
                    )
                    nc.vector.max_index(
                        out=idx_sb[:, t * K:(t + 1) * K],
                        in_max=vals_sb[:, t * K:(t + 1) * K],
                        in_values=sc[:, :],
                    )

            nc.sync.dma_start(
                out=val_out[:, :].rearrange("(t p) k -> p t k", p=P),
                in_=vals_sb[:, :].rearrange("p (t k) -> p t k", k=K),
            )
            nc.sync.dma_start(
                out=idx_out[:, :].rearrange("(t p) k -> p t k", p=P),
                in_=idx_sb[:, :].rearrange("p (t k) -> p t k", k=K),
            )

    return nc


_NO_HOIST = {"InstEventSemaphore"}


def _legalize_matmul_waits(nc, max_waits=1):
    """Walrus codegen rejects engine instructions with >1 embedded sync wait
    ("Too many sync wait commands"). Hoist excess waits onto standalone
    InstEventSemaphore instructions on the same engine just before the
    instruction (engines execute their stream in order, so semantics are
    preserved)."""
    n_fix = 0
    if not hasattr(nc, "_legalize_dummy_sems"):
        # pick sem numbers that no instruction in the program references
        used = set()
        for f in nc.m.functions:
            for blk in f.blocks:
                for inst in blk.instructions:
                    si = inst.sync_info
                    if si is not None:
                        used.update(w.id for w in si.on_wait)
                        used.update(u.id for u in si.on_update)
        engs = (
            mybir.EngineType.PE,
            mybir.EngineType.Activation,
            mybir.EngineType.DVE,
            mybir.EngineType.Pool,
            mybir.EngineType.SP,
        )
        free = [n for n in range(190, 100, -1) if n not in used]
        nc._legalize_dummy_sems = {
            eng: type("S", (), {"num": free[i], "name": f"lg_dummy_{eng.name}"})()
            for i, eng in enumerate(engs)
        }
    for f in nc.m.functions:
        for blk in f.blocks:
            out = []
            for inst in blk.instructions:
                si = inst.sync_info
                if (
                    type(inst).__name__ not in _NO_HOIST
                    and si is not None
                    and len(si.on_wait) > max_waits
                ):
                    waits = list(si.on_wait)
                    keep, hoist = waits[-max_waits:], waits[:-max_waits]
                    dummy = nc._legalize_dummy_sems[inst.engine]
                    for j, w in enumerate(hoist):
                        upd = mybir.SyncUpdate(
                            sync_type="semaphore",
                            id=dummy.num,
                            ant_name=dummy.name,
                            update_mode="sem-inc",
                            update_value=1,
                        )
                        ev = mybir.InstEventSemaphore(
                            name=f"EVW-{inst.name}-{j}",
                            engine=inst.engine,
                            ins=[],
                            outs=[],
                            sync_info=mybir.SyncInfo(on_wait=[w], on_update=[upd]),
                        )
                        out.append(ev)
                    inst.sync_info = mybir.SyncInfo(
                        on_wait=keep, on_update=list(si.on_update)
                    )
                    n_fix += 1
                out.append(inst)
            blk.instructions[:] = out
    return n_fix


def get_program():
    if "nc" not in _prog_cache:
        nc = _build_program()
        _legalize_matmul_waits(nc)
        _prog_cache["nc"] = nc
    return _prog_cache["nc"]


def make_in_maps(x, W1, b1, W2, b2, W3, b3, temperature):
    x = np.ascontiguousarray(np.asarray(x, dtype=np.float32))
    tau = np.exp(np.clip(np.float32(temperature), np.float32(-5.0),
                         np.float32(5.0)), dtype=np.float32)
    negtau = np.full((P, 1), -tau, dtype=np.float32)
    bias = np.stack(
        [np.asarray(b1, np.float32), np.asarray(b2, np.float32),
         np.asarray(b3, np.float32),
         np.float32(-2.0) * np.asarray(b3, np.float32)],
        axis=1,
    )
    bias = np.ascontiguousarray(bias)

    def trunc11(a):
        """Exact f32r split: hi keeps 11 explicit mantissa bits."""
        b = a.astype(np.float32).view(np.uint32)
        return (b & np.uint32(0xFFFFF000)).view(np.float32)

    def hl_stack(a, rev=False):
        hi = trunc11(a)
        lo = (a - hi).astype(np.float32)
        pair = (lo, hi) if rev else (hi, lo)
        return np.ascontiguousarray(np.concatenate(pair, axis=0))

    wts = {}
    for i, W in enumerate((W1, W2, W3)):
        wt = np.asarray(W, np.float32).T
        wts[f"wd{i}"] = hl_stack(wt)
        wts[f"wc{i}"] = hl_stack(wt, rev=True)

    in_maps = []
    xt_all = [np.ascontiguousarray(x[b_i].T) for b_i in range(B)]
    for c in range(NCORES):
        b_i, half = c // 2, c % 2
        xbt = xt_all[b_i]
        if half:
            # roll keys so this core's queries are key columns [0:NQ];
            # local key j holds global key (j + NQ) % N
            xbt = np.ascontiguousarray(np.roll(xbt, -NQ, axis=1))
        in_maps.append({
            "xhl": hl_stack(xbt),
            "bias": bias,
            "negtau": negtau,
            **wts,
        })
    return in_maps


def assemble_outputs(results):
    """results: list of 8 dicts with val_out [NQ,K] f32, idx_out [NQ,K] u32."""
    edges = np.empty((B, N * K, 2), dtype=np.int32)
    logprobs = np.empty((B, N, K), dtype=np.float32)
    rows = np.repeat(np.arange(N, dtype=np.int32), K)
    for b_i in range(B):
        idx = np.concatenate(
            [results[2 * b_i]["idx_out"].astype(np.int32),
             # second half-core saw keys rolled by NQ: un-roll indices
             (results[2 * b_i + 1]["idx_out"].astype(np.int32) + NQ) % N],
            axis=0,
        )
        vals = np.concatenate(
            [results[2 * b_i]["val_out"], results[2 * b_i + 1]["val_out"]], axis=0
        )
        edges[b_i, :, 0] = idx.reshape(-1)
        edges[b_i, :, 1] = rows
        logprobs[b_i] = vals
    return edges, logprobs


def run(inputs, trace=False):
    """Full pipeline; returns ((edges, logprobs), BassKernelResults)."""
    k = int(np.asarray(inputs["k"]))
    assert k == K, f"kernel hardcodes k=8, got {k}"
    nc = get_program()
    in_maps = make_in_maps(
        inputs["x"], inputs["W1"], inputs["b1"], inputs["W2"], inputs["b2"],
        inputs["W3"], inputs["b3"], inputs["temperature"],
    )
    br = run_bass_kernel_spmd(nc, in_maps, list(range(NCORES)), trace=trace)
    return assemble_outputs(br.results), br


def kernel(**inputs):
    (edges, logprobs), _ = run(inputs, trace=False)
    return edges, logprobs


# revision 79
# speedup vs baseline: 1.0895x; 1.0211x over previous
"""DGM kNN kernel for Trainium2 (Bass/Tile), SPMD over 8 NeuronCores.

Problem: 3-layer MLP on x[4,4096,64], pairwise sq-distances per batch,
top-k=8 smallest per row (scaled by tau), outputs (edges, logprobs).

Sharding: core c handles batch c//2, query rows (c%2)*2048 ..+2048.
Each core computes the full-batch MLP (keys) + its query half, the
[2048, 4096] scaled-distance block, and native max8/max_index top-8.
"""

import numpy as np

import concourse.bass as bass
import concourse.mybir as mybir
import concourse.tile as tile
from concourse.bass_utils import run_bass_kernel_spmd

F32 = mybir.dt.float32
F32R = mybir.dt.float32r
U32 = mybir.dt.uint32
AF = mybir.ActivationFunctionType

B, N, D, K = 4, 4096, 64, 8
NCORES = 8
NQ = N // 2          # query rows per core
P = 128
NT = N // P          # 32 x-tiles (keys)
NTQ = NQ // P        # 16 x-tiles (queries) == q tiles
MM_F = 512           # matmul moving free dim (one PSUM bank of fp32)

_prog_cache = {}


def _build_program():
    nc = bass.Bass("TRN2")
    # reserved before TileContext so the numbers never collide with tile sems;
    # used by _legalize_matmul_waits as no-op update targets (one per engine
    # so the race detector sees a single updater per sem)
    nc._legalize_dummy_sems = {
        eng: nc.alloc_semaphore(name=f"legalize_dummy_{eng.name}")
        for eng in (
            mybir.EngineType.PE,
            mybir.EngineType.Activation,
            mybir.EngineType.DVE,
            mybir.EngineType.Pool,
            mybir.EngineType.SP,
        )
    }

    # keys arrive pre-transposed AND rolled per core so that this core's
    # query block is always key columns [0:NQ] (host un-rolls indices).
    # x and W come pre-split into f32r hi/lo pairs (exact decomposition),
    # stacked along the contract dim: [hi(64); lo(64)].
    xhl_d = nc.dram_tensor("xhl", [2 * D, N], F32R, kind="ExternalInput")
    # per layer: diag stack [Whi; Wlo] and cross stack [Wlo; Whi]
    wd_d = [nc.dram_tensor(f"wd{i}", [2 * D, D], F32R, kind="ExternalInput")
            for i in range(3)]
    wc_d = [nc.dram_tensor(f"wc{i}", [2 * D, D], F32R, kind="ExternalInput")
            for i in range(3)]
    # bias columns: b1, b2, b3, -2*b3
    bias_d = nc.dram_tensor("bias", [D, 4], F32, kind="ExternalInput")
    # -tau broadcast per partition
    negtau_d = nc.dram_tensor("negtau", [P, 1], F32, kind="ExternalInput")

    val_out = nc.dram_tensor("val_out", [NQ, K], F32, kind="ExternalOutput")
    idx_out = nc.dram_tensor("idx_out", [NQ, K], U32, kind="ExternalOutput")

    with tile.TileContext(nc) as tc:
        with (
            tc.tile_pool(name="consts", bufs=1) as consts,
            tc.tile_pool(name="mlp", bufs=1) as mlp_pool,
            tc.tile_pool(name="scaled", bufs=3) as sc_pool,
            tc.tile_pool(name="outs", bufs=1) as out_pool,
        ):
            wd, wc = [], []
            for i in range(3):
                w1 = consts.tile([2 * D, D], F32R, tag=f"wd{i}")
                nc.sync.dma_start(out=w1[:, :], in_=wd_d[i][:, :])
                wd.append(w1)
                w2 = consts.tile([2 * D, D], F32R, tag=f"wc{i}")
                nc.sync.dma_start(out=w2[:, :], in_=wc_d[i][:, :])
                wc.append(w2)
            bias_sb = consts.tile([D, 4], F32)
            nc.sync.dma_start(out=bias_sb[:, :], in_=bias_d[:, :])
            negtau = consts.tile([P, 1], F32)
            nc.sync.dma_start(out=negtau[:, :], in_=negtau_d[:, :])

            ones64 = consts.tile([D, 1], F32)
            nc.vector.memset(ones64[:, :], 1.0)
            ones64_2 = consts.tile([D, 2], F32)
            nc.vector.memset(ones64_2[:, :], 1.0)
            ones64r = consts.tile([D, 2], F32R)
            nc.vector.tensor_copy(ones64r[:, :], ones64_2[:, :])

            # ---- x arrives pre-transposed/rolled/hi-lo-stacked: [128, n] ----
            # chunked so layer-1's first sweep starts after 1/8 of the load
            xhl = mlp_pool.tile([2 * D, N], F32R, tag="hl_a")
            for c in range(0, N, MM_F):
                nc.sync.dma_start(
                    out=xhl[:, c:c + MM_F], in_=xhl_d[:, c:c + MM_F]
                )

            # ---- 3-layer MLP via f32r hi/lo sweeps (keys only) ----
            # Intermediate hi/lo stay on lanes 0..63 -- both Whi and Wlo
            # exist at both lane ranges via wd/wc slices, so no partition
            # moves are ever needed.
            rhs65 = mlp_pool.tile([D + 1, N], F32, tag="rhs65")
            hmid = mlp_pool.tile([D, N], F32, tag="hmid")
            lo64 = mlp_pool.tile([D, N], F32R, tag="lo64")
            hl_b = mlp_pool.tile([2 * D, N], F32R, tag="hl_b")

            with tc.tile_pool(name="ps_mlp", bufs=1, space="PSUM") as ps_mlp_pool:
                cur = xhl
                for layer in range(3):
                    last = layer == 2
                    pss = []
                    for stat in (wd[layer], wc[layer]):
                        for ci, c in enumerate(range(0, N, MM_F)):
                            if stat is wd[layer]:
                                ps = ps_mlp_pool.tile([D, MM_F], F32, tag=f"m{ci}")
                                pss.append(ps)
                            nc.tensor.matmul(
                                pss[ci][:, :],
                                lhsT=stat[:, :],
                                rhs=cur[:, c:c + MM_F],
                                start=stat is wd[layer],
                                stop=stat is wc[layer],
                            )
                    dst = rhs65 if last else hmid
                    nxt_hl = hl_b if layer == 0 else xhl
                    # chunk-wise drain + hi/lo re-split so the next layer's
                    # matmuls start while later chunks are still splitting
                    for ci, c in enumerate(range(0, N, MM_F)):
                        sl = slice(c, c + MM_F)
                        nc.scalar.activation(
                            out=dst[0:D, sl],
                            in_=pss[ci][:, :],
                            func=AF.Identity if last else AF.Relu,
                            bias=bias_sb[:, (2 if last else layer):
                                         (3 if last else layer + 1)],
                            scale=1.0,
                        )
                        if not last:
                            nc.scalar.activation(
                                out=nxt_hl[0:D, sl], in_=hmid[:, sl], func=AF.Copy
                            )
                            nc.vector.tensor_sub(
                                lo64[:, sl], hmid[:, sl], nxt_hl[0:D, sl]
                            )
                            nc.sync.dma_start(
                                out=nxt_hl[D:2 * D, sl], in_=lo64[:, sl]
                            )
                    if not last:
                        cur = nxt_hl

            # ---- squared norms (keys; queries are a prefix slice) ----
            # sq = ones @ (htsq_hi + htsq_lo): exact f32r split of h*h, both
            # sweeps share the ones stationary (no weight-reload penalty)
            htsq = mlp_pool.tile([D, N], F32, tag="hmid")     # reuse
            htsq_hi = mlp_pool.tile([D, N], F32R, tag="hl_b")  # reuse
            htsq_lo = mlp_pool.tile([D, N], F32R, tag="lo64")  # reuse
            for c in range(0, N, MM_F):
                sl = slice(c, c + MM_F)
                nc.vector.tensor_mul(htsq[:, sl], rhs65[0:D, sl], rhs65[0:D, sl])
                nc.scalar.activation(
                    out=htsq_hi[:, sl], in_=htsq[:, sl], func=AF.Copy
                )
                nc.vector.tensor_sub(
                    htsq_lo[:, sl], htsq[:, sl], htsq_hi[:, sl]
                )

            sqrow0 = mlp_pool.tile([1, N], F32, tag="hmid")  # htsq fully read
            with tc.tile_pool(name="ps_sq", bufs=1, space="PSUM") as ps_sq_pool:
                pss = []
                for sweep, src in ((0, htsq_hi), (1, htsq_lo)):
                    for ci, c in enumerate(range(0, N, MM_F)):
                        if sweep == 0:
                            ps = ps_sq_pool.tile([2, MM_F], F32, tag=f"sq{ci}")
                            pss.append(ps)
                        nc.tensor.matmul(
                            pss[ci][0:2, :],
                            lhsT=ones64r[:, :],
                            rhs=src[:, c:c + MM_F],
                            start=sweep == 0, stop=sweep == 1,
                        )
                for ci, c in enumerate(range(0, N, MM_F)):
                    nc.scalar.activation(
                        out=sqrow0[0:1, c:c + MM_F],
                        in_=pss[ci][0:1, :],
                        func=AF.Copy,
                    )

            # ---- f32r hi/lo split of h for the fast distance matmuls ----
            # f32r keeps 11 mantissa bits; products of f32r inputs are exact
            # and accumulate in fp32 PSUM, so
            #   G = hi@hi + lo@lo (MM1, diagonal) + lo@hi + hi@lo (MM2, cross)
            # is fp32-class accurate at ~1 cycle/row instead of 8.
            # rhs_b rows: [hi_k (0:64); lo_k (64:128)]  (lo moved by DMA)
            rhs_b = mlp_pool.tile([P, N], F32R, tag="rhs_b")
            lo_tmp = mlp_pool.tile([D, N], F32R, tag="lo64")  # reuse
            for c in range(0, N, MM_F):
                sl = slice(c, c + MM_F)
                nc.vector.tensor_copy(rhs_b[0:D, sl], rhs65[0:D, sl])
                nc.vector.tensor_sub(
                    lo_tmp[:, sl], rhs65[0:D, sl], rhs_b[0:D, sl]
                )
                nc.sync.dma_start(out=rhs_b[D:P, sl], in_=lo_tmp[:, sl])

            # sq2 rows: [sq_hi; sq_lo] on lane 0 (off the critical path --
            # MM3 is the last phase per half)
            sq2 = mlp_pool.tile([2, N], F32R, tag="sq2")
            nc.scalar.activation(out=sq2[0:1, :], in_=sqrow0[0:1, :], func=AF.Copy)
            sqlo = mlp_pool.tile([1, N], F32R, tag="lo64")
            nc.vector.tensor_sub(sqlo[0:1, :], sqrow0[0:1, :], sq2[0:1, :])
            nc.sync.dma_start(out=sq2[1:2, :], in_=sqlo[0:1, :])

            # query-side stationary tiles (queries = key cols [0:NQ]):
            # lhs_1 rows [-2hi_q; -2lo_q], lhs_2 rows [-2lo_q; -2hi_q]
            lhs_1 = mlp_pool.tile([P, NQ], F32R, tag="lhs_1")
            lhs_2 = mlp_pool.tile([P, NQ], F32R, tag="lhs_2")
            for c in range(0, NQ, MM_F):
                sl = slice(c, c + MM_F)
                nc.scalar.activation(
                    out=lhs_1[0:D, sl], in_=rhs_b[0:D, sl],
                    func=AF.Identity, bias=0.0, scale=-2.0,
                )
                nc.scalar.activation(
                    out=lhs_2[0:D, sl], in_=lo_tmp[:, sl],
                    func=AF.Identity, bias=0.0, scale=-2.0,
                )
                nc.sync.dma_start(out=lhs_1[D:P, sl], in_=lhs_2[0:D, sl])
                nc.sync.dma_start(out=lhs_2[D:P, sl], in_=lhs_1[0:D, sl])

            ones2_f = consts.tile([2, P], F32)
            nc.vector.memset(ones2_f[:, :], 1.0)
            ones2 = consts.tile([2, P], F32R)
            nc.vector.tensor_copy(ones2[:, :], ones2_f[:, :])

            # per-query bias: -tau * sq_q where sq_q = sq_keys[0:NQ].
            # SBUF->SBUF partition scatter can't be balanced in one DMA, so
            # bounce through a small DRAM scratch tensor.
            sqq_dram = nc.dram_tensor("sqq_scratch", [NQ], F32, kind="Internal")
            nc.sync.dma_start(out=sqq_dram[:], in_=sqrow0[0:1, 0:NQ])
            sqt = consts.tile([P, NTQ], F32)
            nc.sync.dma_start(
                out=sqt[:, :],
                in_=sqq_dram[:].rearrange("(t p) -> p t", p=P),
            )
            bias_all = consts.tile([P, NTQ], F32)
            nc.vector.tensor_scalar_mul(bias_all[:, :], sqt[:, :], negtau[:, 0:1])

            # ---- distance blocks + top-8 ----
            vals_sb = out_pool.tile([P, NTQ * K], F32)
            idx_sb = out_pool.tile([P, NTQ * K], U32)

            with tc.tile_pool(name="ps_dist", bufs=2, space="PSUM") as ps_dist_pool:
                for t in range(NTQ):
                    sc = sc_pool.tile([P, N], F32, tag="sc")
                    for h in range(2):
                        ph = ps_dist_pool.tile([P, N // 2], F32, tag="dist")
                        # sweep each stationary over all 4 chunks before
                        # switching: weight reloads cost ~450ns, sweeps ~250ns
                        for phase, (lhsT, rhs) in enumerate((
                            (lhs_1[:, t * P:(t + 1) * P], rhs_b),
                            (lhs_2[:, t * P:(t + 1) * P], rhs_b),
                            (ones2[:, :], sq2),
                        )):
                            for i in range(4):
                                off = h * (N // 2) + i * MM_F
                                nc.tensor.matmul(
                                    ph[:, i * MM_F:(i + 1) * MM_F],
                                    lhsT=lhsT,
                                    rhs=rhs[:, off:off + MM_F],
                                    start=phase == 0, stop=phase == 2,
                                )
                        # scaled = -tau * (sq_j - 2G) - tau*sq_q
                        nc.scalar.activation(
                            out=sc[:, h * (N // 2):(h + 1) * (N // 2)],
                            in_=ph[:, :],
                            func=AF.Identity,
                            bias=bias_all[:, t:t + 1],
                            scale=negtau[:, 0:1],
                        )
                    nc.vector.max(
                        out=vals_sb[:, t * K:(t + 1) * K], in_=sc[:, :]
                    )
                    nc.vector.max_index(
                        out=idx_sb[:, t * K:(t + 1) * K],
                        in_max=vals_sb[:, t * K:(t + 1) * K],
                        in_values=sc[:, :],
                    )

            nc.sync.dma_start(
                out=val_out[:, :].rearrange("(t p) k -> p t k", p=P),
                in_=vals_sb[:, :].rearrange("p (t k) -> p t k", k=K),
            )
            nc.sync.dma_start(
                out=idx_out[:, :].rearrange("(t p) k -> p t k", p=P),
                in_=idx_sb[:, :].rearrange("p (t k) -> p t k", k=K),
            )

    return nc


_NO_HOIST = {"InstEventSemaphore"}


def _legalize_matmul_waits(nc, max_waits=1):
    """Walrus codegen rejects engine instructions with >1 embedded sync wait
    ("Too many sync wait commands"). Hoist excess waits onto standalone
    InstEventSemaphore instructions on the same engine just before the
    instruction (engines execute their stream in order, so semantics are
    preserved)."""
    n_fix = 0
    if not hasattr(nc, "_legalize_dummy_sems"):
        # pick sem numbers that no instruction in the program references
        used = set()
        for f in nc.m.functions:
            for blk in f.blocks:
                for inst in blk.instructions:
                    si = inst.sync_info
                    if si is not None:
                        used.update(w.id for w in si.on_wait)
                        used.update(u.id for u in si.on_update)
        engs = (
            mybir.EngineType.PE,
            mybir.EngineType.Activation,
            mybir.EngineType.DVE,
            mybir.EngineType.Pool,
            mybir.EngineType.SP,
        )
        free = [n for n in range(190, 100, -1) if n not in used]
        nc._legalize_dummy_sems = {
            eng: type("S", (), {"num": free[i], "name": f"lg_dummy_{eng.name}"})()
            for i, eng in enumerate(engs)
        }
    for f in nc.m.functions:
        for blk in f.blocks:
            out = []
            for inst in blk.instructions:
                si = inst.sync_info
                if (
                    type(inst).__name__ not in _NO_HOIST
                    and si is not None
                    and len(si.on_wait) > max_waits
                ):
                    waits = list(si.on_wait)
                    keep, hoist = waits[-max_waits:], waits[:-max_waits]
                    dummy = nc._legalize_dummy_sems[inst.engine]
                    for j, w in enumerate(hoist):
                        upd = mybir.SyncUpdate(
                            sync_type="semaphore",
                            id=dummy.num,
                            ant_name=dummy.name,
                            update_mode="sem-inc",
                            update_value=1,
                        )
                        ev = mybir.InstEventSemaphore(
                            name=f"EVW-{inst.name}-{j}",
                            engine=inst.engine,
                            ins=[],
                            outs=[],
                            sync_info=mybir.SyncInfo(on_wait=[w], on_update=[upd]),
                        )
                        out.append(ev)
                    inst.sync_info = mybir.SyncInfo(
                        on_wait=keep, on_update=list(si.on_update)
                    )
                    n_fix += 1
                out.append(inst)
            blk.instructions[:] = out
    return n_fix


def get_program():
    if "nc" not in _prog_cache:
        nc = _build_program()
        _legalize_matmul_waits(nc)
        _prog_cache["nc"] = nc
    return _prog_cache["nc"]


def make_in_maps(x, W1, b1, W2, b2, W3, b3, temperature):
    x = np.ascontiguousarray(np.asarray(x, dtype=np.float32))
    tau = np.exp(np.clip(np.float32(temperature), np.float32(-5.0),
                         np.float32(5.0)), dtype=np.float32)
    negtau = np.full((P, 1), -tau, dtype=np.float32)
    bias = np.stack(
        [np.asarray(b1, np.float32), np.asarray(b2, np.float32),
         np.asarray(b3, np.float32),
         np.float32(-2.0) * np.asarray(b3, np.float32)],
        axis=1,
    )
    bias = np.ascontiguousarray(bias)

    def trunc11(a):
        """Exact f32r split: hi keeps 11 explicit mantissa bits."""
        b = a.astype(np.float32).view(np.uint32)
        return (b & np.uint32(0xFFFFF000)).view(np.float32)

    def hl_stack(a, rev=False):
        hi = trunc11(a)
        lo = (a - hi).astype(np.float32)
        pair = (lo, hi) if rev else (hi, lo)
        return np.ascontiguousarray(np.concatenate(pair, axis=0))

    wts = {}
    for i, W in enumerate((W1, W2, W3)):
        wt = np.asarray(W, np.float32).T
        wts[f"wd{i}"] = hl_stack(wt)
        wts[f"wc{i}"] = hl_stack(wt, rev=True)

    in_maps = []
    xt_all = [np.ascontiguousarray(x[b_i].T) for b_i in range(B)]
    for c in range(NCORES):
        b_i, half = c // 2, c % 2
        xbt = xt_all[b_i]
        if half:
            # roll keys so this core's queries are key columns [0:NQ];
            # local key j holds global key (j + NQ) % N
            xbt = np.ascontiguousarray(np.roll(xbt, -NQ, axis=1))
        in_maps.append({
            "xhl": hl_stack(xbt),
            "bias": bias,
            "negtau": negtau,
            **wts,
        })
    return in_maps


def assemble_outputs(results):
    """results: list of 8 dicts with val_out [NQ,K] f32, idx_out [NQ,K] u32."""
    edges = np.empty((B, N * K, 2), dtype=np.int32)
    logprobs = np.empty((B, N, K), dtype=np.float32)
    rows = np.repeat(np.arange(N, dtype=np.int32), K)
    for b_i in range(B):
        idx = np.concatenate(
            [results[2 * b_i]["idx_out"].astype(np.int32),
             # second half-core saw keys rolled by NQ: un-roll indices
             (results[2 * b_i + 1]["idx_out"].astype(np.int32) + NQ) % N],
            axis=0,
        )
        vals = np.concatenate(
            [results[2 * b_i]["val_out"], results[2 * b_i + 1]["val_out"]], axis=0
        )
        edges[b_i, :, 0] = idx.reshape(-1)
        edges[b_i, :, 1] = rows
        logprobs[b_i] = vals
    return edges, logprobs


def run(inputs, trace=False):
    """Full pipeline; returns ((edges, logprobs), BassKernelResults)."""
    k = int(np.asarray(inputs["k"]))
    assert k == K, f"kernel hardcodes k=8, got {k}"
    nc = get_program()
    in_maps = make_in_maps(
        inputs["x"], inputs["W1"], inputs["b1"], inputs["W2"], inputs["b2"],
        inputs["W3"], inputs["b3"], inputs["temperature"],
    )
    br = run_bass_kernel_spmd(nc, in_maps, list(range(NCORES)), trace=trace)
    return assemble_outputs(br.results), br


def kernel(**inputs):
    (edges, logprobs), _ = run(inputs, trace=False)
    return edges, logprobs
